# revision 1
# baseline (speedup 1.0000x reference)
"""CompoundProteinInteractionPrediction on 8 Trainium2 NeuronCores (Bass/Tile).

Sharding: row-shard the GNN SpMM (each core owns 2048 atoms of A@hs, fed by a
host-transposed fp8 copy of adjacency^T), all-gather the per-layer xs deltas;
sequence-shard the protein CNN with a 15-word halo; replicate the tiny
attention/output weights.  All inputs enter full-size; output is the full
[1, 2] interaction.
"""
import sys

sys.path.insert(0, "/opt/trn_rl_repo")

import numpy as np
import ml_dtypes

import concourse.bass as bass
import concourse.tile as tile
from concourse import bacc, mybir
from concourse.masks import make_identity

F8 = ml_dtypes.float8_e4m3
BF16 = ml_dtypes.bfloat16

DIM = 128
WINDOW = 5
KK = 2 * WINDOW + 1
LAYER_GNN = 3
LAYER_CNN = 3
LAYER_OUT = 2
HALO = WINDOW * LAYER_CNN  # 15

FULL = dict(na=16384, nw=16384, nfp=100000, nword=20000, ncores=8)


def _ceil_div(a, b):
    return (a + b - 1) // b


def _mb_splits(local_a, step=512):
    out = []
    o = 0
    while o < local_a:
        s = min(step, local_a - o)
        out.append((o, s))
        o += s
    return out


def build_kernel(na, nw, nfp, nword, ncores, enable_asserts=False, debug_outs=False,
                 stage=7, timing_reps=1):
    """Build the Bacc program (same program runs SPMD on all cores)."""
    local_a = na // ncores
    local_w = nw // ncores
    ach = na // 128            # atom chunks of 128
    blk = ach // ncores        # chunks per core-block
    lwin = local_w + 2 * HALO  # gathered word window
    wch = _ceil_div(lwin, 128)
    wpad = wch * 128
    mbs = _mb_splits(local_a)

    f32 = mybir.dt.float32
    bf16 = mybir.dt.bfloat16
    f8 = mybir.dt.float8e4
    i32 = mybir.dt.int32
    Relu = mybir.ActivationFunctionType.Relu
    Tanh = mybir.ActivationFunctionType.Tanh
    Ident = mybir.ActivationFunctionType.Identity

    nc = bacc.Bacc("TRN2", target_bir_lowering=False, debug=False,
                   enable_asserts=enable_asserts, num_devices=ncores)

    # ---- DRAM parameters (per-core values supplied via in_maps) ----
    t_bmat = nc.dram_tensor("bmat", [na, local_a], f8, kind="ExternalInput").ap()
    t_etab = nc.dram_tensor("etab", [nfp, DIM], f32, kind="ExternalInput").ap()
    t_wtab = nc.dram_tensor("wtab", [nword + 1, DIM], f32, kind="ExternalInput").ap()
    t_fps = nc.dram_tensor("fps", [128, ach], i32, kind="ExternalInput").ap()
    t_widx = nc.dram_tensor("widx", [128, wch], i32, kind="ExternalInput").ap()
    t_wmask = nc.dram_tensor("wmask", [128, wpad], bf16, kind="ExternalInput").ap()
    t_wgT = nc.dram_tensor("wgT", [DIM, DIM], f32, kind="ExternalInput").ap()
    t_bg = nc.dram_tensor("bg_row", [1, DIM], f32, kind="ExternalInput").ap()
    t_waT = nc.dram_tensor("waT", [DIM, DIM], bf16, kind="ExternalInput").ap()
    t_ba = nc.dram_tensor("ba_col", [DIM, 1], f32, kind="ExternalInput").ap()
    t_convm = nc.dram_tensor("convm", [DIM, KK * DIM], bf16, kind="ExternalInput").ap()
    t_convb = nc.dram_tensor("convb_col", [DIM, 1], f32, kind="ExternalInput").ap()
    t_woT = nc.dram_tensor("woT", [2 * DIM, 2 * DIM], f32, kind="ExternalInput").ap()
    t_bo = nc.dram_tensor("bo_col", [2 * DIM, 1], f32, kind="ExternalInput").ap()
    t_wiT = nc.dram_tensor("wiT", [2 * DIM, 2], f32, kind="ExternalInput").ap()
    t_bi = nc.dram_tensor("bi_col", [2, 1], f32, kind="ExternalInput").ap()
    t_ones = nc.dram_tensor("ones_row", [1, DIM], bf16, kind="ExternalInput").ap()
    t_out = nc.dram_tensor("out", [2, 1], f32, kind="ExternalOutput").ap()
    if debug_outs:
        t_dxs0 = nc.dram_tensor("d_xs0", [128, local_a], f32, kind="ExternalOutput").ap()
        t_dxs1 = nc.dram_tensor("d_xs1", [128, local_a], f32, kind="ExternalOutput").ap()
        t_dcomp = nc.dram_tensor("d_comp", [128, 1], f32, kind="ExternalOutput").ap()
        t_dys = nc.dram_tensor("d_ys", [128, 1], f32, kind="ExternalOutput").ap()
        t_dimg = nc.dram_tensor("d_img", [128, 512], f32, kind="ExternalOutput").ap()
        t_dhsp = nc.dram_tensor("d_hsp", [128, 512], f32, kind="ExternalOutput").ap()

    rg = [list(range(ncores))]

    with tile.TileContext(nc) as tc:
        with (
            tc.tile_pool(name="persist", bufs=1) as persist,
            tc.tile_pool(name="hsp", bufs=24) as hsp,
            tc.tile_pool(name="slabp", bufs=6) as slabp,
            tc.tile_pool(name="gstp", bufs=3) as gstp,
            tc.tile_pool(name="stagep", bufs=2) as stagep,
            tc.tile_pool(name="scrp", bufs=2) as scrp,
            tc.tile_pool(name="dlocp", bufs=2) as dlocp,
            tc.tile_pool(name="smallp", bufs=10) as smallp,
            tc.tile_pool(name="catp", bufs=3) as catp,
            tc.tile_pool(name="dram", bufs=1, space="DRAM") as dram,
            tc.tile_pool(name="ps_spmm", bufs=1, space="PSUM") as ps_spmm,
            tc.tile_pool(name="ps_misc", bufs=2, space="PSUM") as ps_misc,
            tc.tile_pool(name="ps_conv", bufs=2, space="PSUM") as ps_conv,
        ):
            # ---- small weights into SBUF (unique tags: default tag aliases!) ----
            wgT = persist.tile([DIM, DIM], f32, tag="wgT")
            bg_row = persist.tile([1, DIM], f32, tag="bg_row")
            waT = persist.tile([DIM, DIM], bf16, tag="waT")
            ba_col = persist.tile([DIM, 1], f32, tag="ba_col")
            convm = persist.tile([DIM, KK * DIM], bf16, tag="convm")
            convb_col = persist.tile([DIM, 1], f32, tag="convb_col")
            woT_sb = persist.tile([DIM, 4 * DIM], f32, tag="woT_sb")
            bo_sb = persist.tile([DIM, 2], f32, tag="bo_sb")
            wiT_sb = persist.tile([DIM, 4], f32, tag="wiT_sb")
            bi_sb = persist.tile([2, 1], f32, tag="bi_sb")
            ones_row = persist.tile([1, DIM], bf16, tag="ones_row")
            ident = persist.tile([DIM, DIM], f32, tag="ident")
            ones_f32 = persist.tile([1, DIM], f32, tag="ones_f32")
            fps_sb = persist.tile([128, ach], i32, tag="fps_sb")
            widx_sb = persist.tile([128, wch], i32, tag="widx_sb")
            wmask_sb = persist.tile([128, wpad], bf16, tag="wmask_sb")

            nc.sync.dma_start(wgT[:], t_wgT[:])
            nc.sync.dma_start(bg_row[:], t_bg[:])
            nc.sync.dma_start(waT[:], t_waT[:])
            nc.sync.dma_start(ba_col[:], t_ba[:])
            nc.sync.dma_start(convm[:], t_convm[:])
            nc.sync.dma_start(convb_col[:], t_convb[:])
            for j in range(2):
                for i in range(2):
                    nc.sync.dma_start(
                        woT_sb[:, (j * 2 + i) * DIM:(j * 2 + i + 1) * DIM],
                        t_woT[j * DIM:(j + 1) * DIM, i * DIM:(i + 1) * DIM])
                nc.sync.dma_start(bo_sb[:, j:j + 1], t_bo[j * DIM:(j + 1) * DIM, :])
                nc.sync.dma_start(wiT_sb[:, 2 * j:2 * j + 2], t_wiT[j * DIM:(j + 1) * DIM, :])
            nc.sync.dma_start(bi_sb[:], t_bi[:])
            nc.sync.dma_start(ones_row[:], t_ones[:])
            nc.sync.dma_start(fps_sb[:], t_fps[:])
            nc.sync.dma_start(widx_sb[:], t_widx[:])
            nc.sync.dma_start(wmask_sb[:], t_wmask[:])
            make_identity(nc, ident[:])
            nc.gpsimd.memset(ones_f32[:], 1.0)

            # ---- persistent GNN state: 8 blocks of [128, local_a] ----
            xsT = [persist.tile([128, local_a], f32, tag=f"xsT{b}", name=f"xsT{b}")
                   for b in range(ncores)]

            # ---- fingerprint gather + transpose into dim-major xs ----
            for g in range(0, ach, 8):
                bs = min(8, ach - g)
                gt = gstp.tile([128, 8 * DIM], f32, tag="gst")
                for j in range(bs):
                    # multi-index indirect DMA silently fails on HW; one col each
                    nc.gpsimd.indirect_dma_start(
                        out=gt[:, j * DIM:(j + 1) * DIM],
                        out_offset=None,
                        in_=t_etab[:],
                        in_offset=bass.IndirectOffsetOnAxis(
                            ap=fps_sb[:, g + j:g + j + 1], axis=0),
                    )
                for j in range(bs):
                    ci = g + j
                    b, o = divmod(ci, blk)
                    pt = ps_misc.tile([128, 128], f32, tag="pmisc")
                    nc.tensor.transpose(pt[:], gt[:, j * DIM:(j + 1) * DIM], ident[:])
                    nc.vector.tensor_copy(xsT[b][:, o * 128:(o + 1) * 128], pt[:])

            bmat_r = t_bmat.rearrange("(t p) m -> t p m", p=128)

            ag_in = []
            ag_out = []
            for l in range(LAYER_GNN - 1):
                ag_in.append(dram.tile([128, local_a], f32, tag=f"agi{l}",
                                       name=f"agi{l}"))
                ag_out.append(dram.tile([128 * ncores, local_a], f32,
                                        tag=f"ago{l}", name=f"ago{l}",
                                        addr_space="Shared"))
            arc_in = dram.tile([128, 1], f32, tag="arci")
            arc_out = dram.tile([128, 1], f32, tag="arco", addr_space="Shared")
            arp_in = dram.tile([128, 1], f32, tag="arpi")
            arp_out = dram.tile([128, 1], f32, tag="arpo", addr_space="Shared")

            # protein tiles
            imgA = persist.tile([128, wpad], bf16, tag="imgA")
            imgB = persist.tile([128, wpad], bf16, tag="imgB")
            hs_pT = persist.tile([128, local_w], bf16, tag="hspT")
            w_bf = persist.tile([1, local_w], bf16, tag="wbf")
            comp_sum = persist.tile([128, 1], f32, tag="csum")
            ys_ar = persist.tile([128, 1], f32, tag="ysar")

            def gnn_layer(layer, do_comm=True):
                psums = [ps_spmm.tile([128, ms], f32, tag=f"spmm{mb}",
                                      name=f"spmm_l{layer}_{mb}")
                         for mb, (mo, ms) in enumerate(mbs)]
                for ki in range(ach):
                    # hs chunk ki = relu(xs[chunk] @ Wg.T + bg), via bias-matmul trick
                    b, o = divmod(ki, blk)
                    hp = ps_misc.tile([128, 128], f32, tag="pmisc")
                    nc.tensor.matmul(hp[:], ones_f32[:], bg_row[:], start=True, stop=False)
                    nc.tensor.matmul(hp[:], xsT[b][:, o * 128:(o + 1) * 128], wgT[:],
                                     start=False, stop=True)
                    hch = hsp.tile([128, 128], bf16, tag="hs")
                    nc.scalar.activation(hch[:], hp[:], Relu)
                    # adjacency slab (two k-tiles per DMA)
                    if ki % 2 == 0:
                        slab = slabp.tile([128, 2 * local_a], f8, tag="slab")
                        nc.sync.dma_start(slab[:, :local_a], bmat_r[ki])
                        nc.sync.dma_start(slab[:, local_a:], bmat_r[ki + 1])
                    part = slab[:, (ki % 2) * local_a:(ki % 2 + 1) * local_a]
                    for mb, (mo, ms) in enumerate(mbs):
                        nc.tensor.matmul(psums[mb][:], hch[:], part[:, mo:mo + ms],
                                         start=(ki == 0), stop=(ki == ach - 1))
                if layer < LAYER_GNN - 1:
                    dloc = dlocp.tile([128, local_a], f32, tag="dloc")
                    for mb, (mo, ms) in enumerate(mbs):
                        nc.vector.tensor_copy(dloc[:, mo:mo + ms], psums[mb][:])
                    if not do_comm:
                        return
                    nc.sync.dma_start(ag_in[layer][:], dloc[:])
                    nc.gpsimd.collective_compute(
                        "AllGather", mybir.AluOpType.bypass,
                        ins=[ag_in[layer][:].opt()], outs=[ag_out[layer][:].opt()],
                        replica_groups=rg)
                    for c in range(ncores):
                        st = stagep.tile([128, local_a], f32, tag="agst")
                        nc.sync.dma_start(st[:], ag_out[layer][c * 128:(c + 1) * 128, :])
                        nc.vector.tensor_tensor(xsT[c][:], xsT[c][:], st[:],
                                                op=mybir.AluOpType.add)
                else:
                    # final layer: only the atom-mean of xs3 is needed.
                    # AllReduce the per-core delta3 partial; the xs2 sum is
                    # replicated on every core, so add it locally afterwards.
                    acc = smallp.tile([128, 1], f32, tag="small")
                    tmp = smallp.tile([128, 1], f32, tag="small")
                    xs2s = smallp.tile([128, 1], f32, tag="small")
                    nc.vector.reduce_sum(acc[:], psums[0][:], axis=mybir.AxisListType.X)
                    for mb in range(1, len(mbs)):
                        nc.vector.reduce_sum(tmp[:], psums[mb][:], axis=mybir.AxisListType.X)
                        nc.vector.tensor_tensor(acc[:], acc[:], tmp[:], op=mybir.AluOpType.add)
                    nc.vector.reduce_sum(xs2s[:], xsT[0][:], axis=mybir.AxisListType.X)
                    for c in range(1, ncores):
                        nc.vector.reduce_sum(tmp[:], xsT[c][:], axis=mybir.AxisListType.X)
                        nc.vector.tensor_tensor(xs2s[:], xs2s[:], tmp[:], op=mybir.AluOpType.add)
                    nc.sync.dma_start(arc_in[:], acc[:])
                    nc.gpsimd.collective_compute(
                        "AllReduce", mybir.AluOpType.add,
                        ins=[arc_in[:].opt()], outs=[arc_out[:].opt()],
                        replica_groups=rg)
                    nc.sync.dma_start(comp_sum[:], arc_out[:])
                    nc.vector.tensor_tensor(comp_sum[:], comp_sum[:], xs2s[:],
                                            op=mybir.AluOpType.add)

            def protein_gather():
                # word-embedding gather (halo + out-of-range -> zero row) + transpose
                for g in range(0, wch, 8):
                    bs = min(8, wch - g)
                    gt = gstp.tile([128, 8 * DIM], f32, tag="gst")
                    for j in range(bs):
                        nc.gpsimd.indirect_dma_start(
                            out=gt[:, j * DIM:(j + 1) * DIM],
                            out_offset=None,
                            in_=t_wtab[:],
                            in_offset=bass.IndirectOffsetOnAxis(
                                ap=widx_sb[:, g + j:g + j + 1], axis=0),
                        )
                    for j in range(bs):
                        wi = g + j
                        pt = ps_misc.tile([128, 128], f32, tag="pmisc")
                        nc.tensor.transpose(pt[:], gt[:, j * DIM:(j + 1) * DIM], ident[:])
                        nc.vector.tensor_copy(imgA[:, wi * 128:(wi + 1) * 128], pt[:])

            def protein_conv():
                # 3 conv layers as banded matmuls; imgA -> imgB -> imgA -> imgB
                bufs = [imgA, imgB]
                for l in range(LAYER_CNN):
                    lo = WINDOW * (l + 1)
                    hi = lwin - WINDOW * (l + 1)
                    src, dst = bufs[l % 2], bufs[(l + 1) % 2]
                    o = lo
                    while o < hi:
                        ms = min(512, hi - o)
                        pc = ps_conv.tile([128, 512], f32, tag="pconv")
                        for a in range(KK):
                            nc.tensor.matmul(
                                pc[:, :ms], convm[:, a * DIM:(a + 1) * DIM],
                                src[:, o + a - WINDOW:o + a - WINDOW + ms],
                                start=(a == 0), stop=(a == KK - 1))
                        nc.scalar.activation(dst[:, o:o + ms], pc[:, :ms], Relu,
                                             bias=convb_col[:])
                        o += ms
                    if l < LAYER_CNN - 1:
                        # zero out-of-sequence edge columns (global conv zero-padding)
                        nc.vector.tensor_tensor(dst[:, lo:hi], dst[:, lo:hi],
                                                wmask_sb[:, lo:hi],
                                                op=mybir.AluOpType.mult)
                # hs_p = relu(xs_p @ Wa.T + ba), dim-major
                xsp = bufs[LAYER_CNN % 2]
                o = 0
                while o < local_w:
                    ms = min(512, local_w - o)
                    pc = ps_conv.tile([128, 512], f32, tag="pconv")
                    nc.tensor.matmul(pc[:, :ms], waT[:], xsp[:, HALO + o:HALO + o + ms],
                                     start=True, stop=True)
                    nc.scalar.activation(hs_pT[:, o:o + ms], pc[:, :ms], Relu,
                                         bias=ba_col[:])
                    o += ms

            def tail():
                # h = relu(Wa @ compound + ba); compound = comp_sum / na
                comp_bf = smallp.tile([128, 1], bf16, tag="smallbf")
                nc.vector.tensor_scalar_mul(comp_bf[:], comp_sum[:], 1.0 / na)
                ph = ps_misc.tile([128, 128], f32, tag="pmisc")
                nc.tensor.matmul(ph[:, :1], waT[:], comp_bf[:], start=True, stop=True)
                h_bf = smallp.tile([128, 1], bf16, tag="smallbf")
                nc.scalar.activation(h_bf[:], ph[:, :1], Relu, bias=ba_col[:])
                if stage == 71:
                    return
                # w = tanh(h . hs_p)
                o = 0
                while o < local_w:
                    ms = min(512, local_w - o)
                    pw = ps_misc.tile([128, 512], f32, tag="pmisc")
                    nc.tensor.matmul(pw[:1, :ms], h_bf[:], hs_pT[:, o:o + ms],
                                     start=True, stop=True)
                    nc.scalar.activation(w_bf[:, o:o + ms], pw[:1, :ms], Tanh)
                    o += ms
                if stage == 72:
                    return
                # ys partial = sum_l w[l] * hs_p[:, l]
                # (tensor_tensor_reduce hangs on HW here; use mult + reduce_sum)
                prev = None
                o = 0
                while o < local_w:
                    ms = min(512, local_w - o)
                    pb = ps_misc.tile([128, 512], f32, tag="pmisc")
                    nc.tensor.matmul(pb[:, :ms], ones_row[:], w_bf[:, o:o + ms],
                                     start=True, stop=True)
                    scr = scrp.tile([128, 512], f32, tag="scr")
                    nc.vector.tensor_tensor(scr[:, :ms], pb[:, :ms],
                                            hs_pT[:, o:o + ms],
                                            op=mybir.AluOpType.mult)
                    acc = smallp.tile([128, 1], f32, tag="small")
                    nc.vector.reduce_sum(acc[:], scr[:, :ms], axis=mybir.AxisListType.X)
                    if prev is not None:
                        nc.vector.tensor_tensor(acc[:], acc[:], prev[:],
                                                op=mybir.AluOpType.add)
                    prev = acc
                    o += ms
                if stage == 73:
                    return
                nc.sync.dma_start(arp_in[:], prev[:])
                nc.gpsimd.collective_compute(
                    "AllReduce", mybir.AluOpType.add,
                    ins=[arp_in[:].opt()], outs=[arp_out[:].opt()],
                    replica_groups=rg)
                nc.sync.dma_start(ys_ar[:], arp_out[:])
                if stage == 74:
                    return
                # output MLP on cat = [compound, protein]
                cat = catp.tile([128, 2], f32, tag="cat")
                nc.vector.tensor_scalar_mul(cat[:, 0:1], comp_sum[:], 1.0 / na)
                nc.vector.tensor_scalar_mul(cat[:, 1:2], ys_ar[:], 1.0 / nw)
                for l in range(LAYER_OUT):
                    ncat = catp.tile([128, 2], f32, tag="cat")
                    for i in range(2):
                        pm = ps_misc.tile([128, 128], f32, tag="pmisc")
                        for j in range(2):
                            nc.tensor.matmul(pm[:, :1],
                                             woT_sb[:, (j * 2 + i) * DIM:(j * 2 + i + 1) * DIM],
                                             cat[:, j:j + 1],
                                             start=(j == 0), stop=(j == 1))
                        nc.scalar.activation(ncat[:, i:i + 1], pm[:, :1], Relu,
                                             bias=bo_sb[:, i:i + 1])
                    cat = ncat
                pf = ps_misc.tile([128, 128], f32, tag="pmisc")
                for j in range(2):
                    nc.tensor.matmul(pf[:2, :1], wiT_sb[:, 2 * j:2 * j + 2],
                                     cat[:, j:j + 1], start=(j == 0), stop=(j == 1))
                res = smallp.tile([2, 1], f32, tag="res")
                nc.scalar.activation(res[:], pf[:2, :1], Ident, bias=bi_sb[:])
                nc.sync.dma_start(t_out[:], res[:])

            def finish_early():
                res2 = smallp.tile([2, 1], f32, tag="res")
                nc.vector.tensor_copy(res2[:], bi_sb[:])
                nc.sync.dma_start(t_out[:], res2[:])

            protein_gather()
            if debug_outs:
                nc.sync.dma_start(t_dxs0[:], xsT[0][:])
            if stage >= 2:
                gnn_layer(0, do_comm=(stage >= 3))
            if stage >= 4:
                protein_conv()
            for _rep in range(timing_reps - 1):
                # timing-only extra GNN layer pairs (semantically garbage);
                # collectives need fresh Shared buffers each time
                for _l in range(LAYER_GNN - 1):
                    ag_in[_l] = dram.tile([128, local_a], f32,
                                          tag=f"agi_r{_rep}_{_l}", name=f"agi_r{_rep}_{_l}")
                    ag_out[_l] = dram.tile([128 * ncores, local_a], f32,
                                           tag=f"ago_r{_rep}_{_l}", name=f"ago_r{_rep}_{_l}",
                                           addr_space="Shared")
                gnn_layer(0)
                gnn_layer(1)
            if debug_outs:
                nc.sync.dma_start(t_dxs1[:], xsT[0][:])
                ims = min(512, wpad)
                dimg_bf = persist.tile([128, 512], f32, tag="dimg")
                nc.scalar.copy(dimg_bf[:, :ims], imgA[:, :ims])
                nc.sync.dma_start(t_dimg[:, :ims], dimg_bf[:, :ims])
                dhsp_bf = persist.tile([128, 512], f32, tag="dhsp")
                hms = min(512, local_w)
                nc.scalar.copy(dhsp_bf[:, :hms], hs_pT[:, :hms])
                nc.sync.dma_start(t_dhsp[:, :hms], dhsp_bf[:, :hms])
            if stage >= 5:
                gnn_layer(1)
            if stage >= 6:
                gnn_layer(2)
            if debug_outs:
                nc.sync.dma_start(t_dcomp[:], comp_sum[:])
            if stage >= 7 or stage >= 70:
                tail()
            if stage < 7 or (70 <= stage <= 74):
                finish_early()
            if debug_outs:
                nc.sync.dma_start(t_dys[:], ys_ar[:])

    nc.compile()
    return nc


def prep_in_maps(inputs, na, nw, nfp, nword, ncores):
    """Host-side sharding/layout prep. Lossless casts only (A is 0/1 -> fp8)."""
    local_a = na // ncores
    local_w = nw // ncores
    ach = na // 128
    lwin = local_w + 2 * HALO
    wch = _ceil_div(lwin, 128)
    wpad = wch * 128

    fingerprints = np.asarray(inputs["fingerprints"]).astype(np.int32)
    adjacency = np.asarray(inputs["adjacency"], dtype=np.float32)
    words = np.asarray(inputs["words"]).astype(np.int32)
    embed_fp = np.ascontiguousarray(np.asarray(inputs["embed_fp"], dtype=np.float32))
    embed_word = np.asarray(inputs["embed_word"], dtype=np.float32)
    Wg = np.asarray(inputs["Wg"], dtype=np.float32)
    bg = np.asarray(inputs["bg"], dtype=np.float32)
    conv_k = np.asarray(inputs["conv_k"], dtype=np.float32)
    conv_b = np.asarray(inputs["conv_b"], dtype=np.float32)
    Wa = np.asarray(inputs["Wa"], dtype=np.float32)
    ba = np.asarray(inputs["ba"], dtype=np.float32)
    Wo = np.asarray(inputs["Wo"], dtype=np.float32)
    bo = np.asarray(inputs["bo"], dtype=np.float32)
    Wi = np.asarray(inputs["Wi"], dtype=np.float32)
    bi = np.asarray(inputs["bi"], dtype=np.float32)

    # adjacency -> fp8 bit pattern (0.0 -> 0x00, nonzero -> 0x38 = 1.0 in e4m3)
    a8 = (adjacency != 0).astype(np.uint8) * np.uint8(0x38)

    wtab = np.concatenate([embed_word, np.zeros((1, DIM), np.float32)], axis=0)
    wtab = np.ascontiguousarray(wtab)

    fps_l = np.ascontiguousarray(fingerprints.reshape(ach, 128).T.astype(np.int32))

    K2 = conv_k[0, 0]  # [11, 11]
    M = np.zeros((DIM, KK * DIM), np.float32)
    for a in range(KK):
        Ma = np.zeros((DIM, DIM), np.float32)
        for b_ in range(KK):
            Ma += K2[a, b_] * np.eye(DIM, k=5 - b_, dtype=np.float32)
        M[:, a * DIM:(a + 1) * DIM] = Ma

    common = dict(
        etab=embed_fp,
        wtab=wtab,
        fps=fps_l,
        wgT=np.ascontiguousarray(Wg.T).astype(np.float32),
        bg_row=bg.reshape(1, DIM).astype(np.float32),
        waT=np.ascontiguousarray(Wa.T).astype(BF16),
        ba_col=ba.reshape(DIM, 1).astype(np.float32),
        convm=M.astype(BF16),
        convb_col=np.full((DIM, 1), conv_b[0], np.float32),
        woT=np.ascontiguousarray(Wo.T).astype(np.float32),
        bo_col=bo.reshape(2 * DIM, 1).astype(np.float32),
        wiT=np.ascontiguousarray(Wi.T).astype(np.float32),
        bi_col=bi.reshape(2, 1).astype(np.float32),
        ones_row=np.ones((1, DIM), BF16),
    )

    in_maps = []
    for c in range(ncores):
        sl = slice(c * local_a, (c + 1) * local_a)
        bmat = np.ascontiguousarray(a8[sl, :].T).view(F8)
        w0 = c * local_w - HALO
        pos = np.arange(wpad)
        gidx = w0 + pos
        valid = (gidx >= 0) & (gidx < nw) & (pos < lwin)
        widx = np.where(valid, words[np.clip(gidx, 0, nw - 1)], nword).astype(np.int32)
        widx_l = np.ascontiguousarray(widx.reshape(wch, 128).T)
        wmask = np.broadcast_to(
            ((gidx >= 0) & (gidx < nw)).astype(BF16)[None, :], (128, wpad))
        m = dict(common)
        m.update(bmat=bmat, widx=widx_l, wmask=np.ascontiguousarray(wmask))
        in_maps.append(m)
    return in_maps


_CACHE = {}


def _get_kernel(cfg_key):
    if cfg_key not in _CACHE:
        na, nw, nfp, nword, ncores = cfg_key
        _CACHE[cfg_key] = build_kernel(na, nw, nfp, nword, ncores)
    return _CACHE[cfg_key]


def kernel(**inputs) -> np.ndarray:
    from concourse import bass_utils
    cfg = FULL
    key = (cfg["na"], cfg["nw"], cfg["nfp"], cfg["nword"], cfg["ncores"])
    nc = _get_kernel(key)
    in_maps = prep_in_maps(inputs, *key)
    res = bass_utils.run_bass_kernel_spmd(
        nc, in_maps, core_ids=list(range(cfg["ncores"])), trace=False)
    out = np.asarray(res.results[0]["out"], np.float32).reshape(1, 2)
    return out



# revision 4
# speedup vs baseline: 11.8038x; 11.8038x over previous
"""CompoundProteinInteractionPrediction on 8 Trainium2 NeuronCores (Bass/Tile).

v2 design.  The v1 bottleneck was ~20ms of gpsimd indirect-DMA embedding
gathers (~1.1us per gathered row).  v2 replaces both gathers with one-hot
matmul gathers: the embedding table streams through SBUF contiguously as
matmul lhsT tiles while a one-hot rhs is built on-chip (DVE tensor_scalar
subtract+is_equal against an iota column).  Each core's atoms are sorted by
fingerprint on the host so the one-hot only needs a narrow fixed window of
output columns per 128-row table chunk; the atom permutation is absorbed
into the host-side relayout of the adjacency, so no on-device unsort is
needed for the GNN.  The protein word gather uses the same trick plus a
small on-device unsort matmul (conv needs sequence order).

GNN sharding: xs lives row-sharded (2048 atoms/core, fp-sorted order);
per layer each core computes its own hs chunk atom-major, AllGathers it
(bf16, 0.5MB), and runs the SpMM against its fp8 adjacency column block.
"""
import sys

sys.path.insert(0, "/opt/trn_rl_repo")

import numpy as np
import ml_dtypes

import concourse.bass as bass
import concourse.tile as tile
from concourse import bacc, mybir
from concourse.masks import make_identity

F8 = ml_dtypes.float8_e4m3
BF16 = ml_dtypes.bfloat16

DIM = 128
WINDOW = 5
KK = 2 * WINDOW + 1
LAYER_GNN = 3
LAYER_CNN = 3
LAYER_OUT = 2
HALO = WINDOW * LAYER_CNN  # 15

FULL = dict(na=16384, nw=16384, nfp=100000, nword=20000, ncores=8)

W_F = 192    # one-hot window width, fingerprint gather (max required: 128)
W_W = 384    # one-hot window width, word gather (max required: 320)


def _ceil_div(a, b):
    return (a + b - 1) // b


def _win_starts(n_chunks, n_loc, t_real, W):
    """Compile-time one-hot window start per 128-row table chunk."""
    out = []
    for k in range(n_chunks):
        exp = n_loc * (128 * k + 64) / t_real
        out.append(int(np.clip(round(exp - W / 2), 0, n_loc - W)))
    return out


def _segments(ck, W, n_loc):
    """Split window [ck, ck+W) at 512-col PSUM-bank boundaries."""
    segs = []
    lo = ck
    end = min(ck + W, n_loc)
    while lo < end:
        b = lo // 512
        hi = min(end, (b + 1) * 512)
        segs.append((b, lo, hi))
        lo = hi
    return segs


def build_kernel(na, nw, nfp, nword, ncores, enable_asserts=False, stage=7):
    local_a = na // ncores
    local_w = nw // ncores
    lwin = local_w + 2 * HALO      # gathered word window
    wch = _ceil_div(lwin, 128)
    wpad = wch * 128               # 2176
    tpad_f = _ceil_div(nfp, 128) * 128    # 100096
    tpad_w = _ceil_div(nword + 1, 128) * 128  # 20096
    nch_f = tpad_f // 128          # 782
    nch_w = tpad_w // 128          # 157
    fstarts = _win_starts(nch_f, local_a, nfp, W_F)
    wstarts = _win_starts(nch_w, wpad, nword, W_W)
    nblk_a = local_a // 512        # 4
    nblk_w = _ceil_div(wpad, 512)  # 5 (last block 128 wide)

    f32 = mybir.dt.float32
    bf16 = mybir.dt.bfloat16
    f8 = mybir.dt.float8e4
    Relu = mybir.ActivationFunctionType.Relu
    Tanh = mybir.ActivationFunctionType.Tanh
    Ident = mybir.ActivationFunctionType.Identity
    SUB = mybir.AluOpType.subtract
    EQ = mybir.AluOpType.is_equal
    ADD = mybir.AluOpType.add
    MUL = mybir.AluOpType.mult

    nc = bacc.Bacc("TRN2", target_bir_lowering=False, debug=False,
                   enable_asserts=enable_asserts, num_devices=ncores)

    # ---- DRAM parameters (per-core values supplied via in_maps) ----
    t_bmat = nc.dram_tensor("bmat", [na, local_a], f8, kind="ExternalInput").ap()
    t_etab = nc.dram_tensor("etabpm", [128, tpad_f], bf16, kind="ExternalInput").ap()
    t_wtab = nc.dram_tensor("wtabpm", [128, tpad_w], bf16, kind="ExternalInput").ap()
    t_sfp = nc.dram_tensor("sfp_row", [1, local_a], f32, kind="ExternalInput").ap()
    t_sw = nc.dram_tensor("sw_row", [1, wpad], f32, kind="ExternalInput").ap()
    t_winv = nc.dram_tensor("winv_row", [1, wpad], f32, kind="ExternalInput").ap()
    t_iota = nc.dram_tensor("iota_col", [128, 1], f32, kind="ExternalInput").ap()
    t_wgT = nc.dram_tensor("wgT", [DIM, DIM], f32, kind="ExternalInput").ap()
    t_bg = nc.dram_tensor("bg_row", [1, DIM], f32, kind="ExternalInput").ap()
    t_waT = nc.dram_tensor("waT", [DIM, DIM], bf16, kind="ExternalInput").ap()
    t_ba = nc.dram_tensor("ba_col", [DIM, 1], f32, kind="ExternalInput").ap()
    t_convm = nc.dram_tensor("convm", [DIM, KK * DIM], bf16, kind="ExternalInput").ap()
    t_convb = nc.dram_tensor("convb_col", [DIM, 1], f32, kind="ExternalInput").ap()
    t_woT = nc.dram_tensor("woT", [2 * DIM, 2 * DIM], f32, kind="ExternalInput").ap()
    t_bo = nc.dram_tensor("bo_col", [2 * DIM, 1], f32, kind="ExternalInput").ap()
    t_wiT = nc.dram_tensor("wiT", [2 * DIM, 2], f32, kind="ExternalInput").ap()
    t_bi = nc.dram_tensor("bi_col", [2, 1], f32, kind="ExternalInput").ap()
    t_ones = nc.dram_tensor("ones_row", [1, DIM], bf16, kind="ExternalInput").ap()
    t_wmask = nc.dram_tensor("wmask", [128, wpad], bf16, kind="ExternalInput").ap()
    t_out = nc.dram_tensor("out", [2, 1], f32, kind="ExternalOutput").ap()

    rg = [list(range(ncores))]

    with tile.TileContext(nc) as tc:
        with (
            tc.tile_pool(name="persist", bufs=1) as persist,
            tc.tile_pool(name="tabp", bufs=4) as tabp,
            tc.tile_pool(name="ohp", bufs=4) as ohp,
            tc.tile_pool(name="ohup", bufs=2) as ohup,
            tc.tile_pool(name="slabp", bufs=6) as slabp,
            tc.tile_pool(name="hsamp", bufs=2) as hsamp,
            tc.tile_pool(name="smallp", bufs=10) as smallp,
            tc.tile_pool(name="catp", bufs=3) as catp,
            tc.tile_pool(name="scrp", bufs=2) as scrp,
            tc.tile_pool(name="dram", bufs=1, space="DRAM") as dram,
            tc.tile_pool(name="ps4", bufs=1, space="PSUM") as ps4,
            tc.tile_pool(name="ps_x", bufs=2, space="PSUM") as ps_x,
            tc.tile_pool(name="ps_s", bufs=2, space="PSUM") as ps_s,
        ):
            # ---- small weights into SBUF ----
            wgT = persist.tile([DIM, DIM], f32, tag="wgT")
            bg_row = persist.tile([1, DIM], f32, tag="bg_row")
            waT = persist.tile([DIM, DIM], bf16, tag="waT")
            ba_col = persist.tile([DIM, 1], f32, tag="ba_col")
            convm = persist.tile([DIM, KK * DIM], bf16, tag="convm")
            convb_col = persist.tile([DIM, 1], f32, tag="convb_col")
            woT_sb = persist.tile([DIM, 4 * DIM], f32, tag="woT_sb")
            bo_sb = persist.tile([DIM, 2], f32, tag="bo_sb")
            wiT_sb = persist.tile([DIM, 4], f32, tag="wiT_sb")
            bi_sb = persist.tile([2, 1], f32, tag="bi_sb")
            ones_row = persist.tile([1, DIM], bf16, tag="ones_row")
            iota_col = persist.tile([128, 1], f32, tag="iota_col")
            wmask_sb = persist.tile([128, wpad], bf16, tag="wmask_sb")
            ident_bf = persist.tile([DIM, DIM], bf16, tag="ident_bf")
            ones_f32 = persist.tile([1, DIM], f32, tag="ones_f32")
            zcol = persist.tile([1, DIM], bf16, tag="zcol")
            zrow = persist.tile([1, 512], bf16, tag="zrow")

            nc.sync.dma_start(wgT[:], t_wgT[:])
            nc.sync.dma_start(bg_row[:], t_bg[:])
            nc.sync.dma_start(waT[:], t_waT[:])
            nc.sync.dma_start(ba_col[:], t_ba[:])
            nc.sync.dma_start(convm[:], t_convm[:])
            nc.sync.dma_start(convb_col[:], t_convb[:])
            for j in range(2):
                for i in range(2):
                    nc.sync.dma_start(
                        woT_sb[:, (j * 2 + i) * DIM:(j * 2 + i + 1) * DIM],
                        t_woT[j * DIM:(j + 1) * DIM, i * DIM:(i + 1) * DIM])
                nc.sync.dma_start(bo_sb[:, j:j + 1], t_bo[j * DIM:(j + 1) * DIM, :])
                nc.sync.dma_start(wiT_sb[:, 2 * j:2 * j + 2], t_wiT[j * DIM:(j + 1) * DIM, :])
            nc.sync.dma_start(bi_sb[:], t_bi[:])
            nc.sync.dma_start(ones_row[:], t_ones[:])
            nc.sync.dma_start(iota_col[:], t_iota[:])
            nc.sync.dma_start(wmask_sb[:], t_wmask[:])
            make_identity(nc, ident_bf[:])
            nc.gpsimd.memset(ones_f32[:], 1.0)
            nc.gpsimd.memset(zcol[:], 0.0)
            nc.gpsimd.memset(zrow[:], 0.0)

            # ---- persistent state ----
            xs = persist.tile([128, local_a], f32, tag="xs")           # dim-major
            hs_all = [persist.tile([128, local_a], bf16, tag=f"hsall{r}",
                                   name=f"hsall{r}")
                      for r in range(ncores)]
            sfp_bc = persist.tile([128, local_a], f32, tag="sfp_bc")
            sw_bc = persist.tile([128, wpad], f32, tag="sw_bc")
            winv_bc = persist.tile([128, wpad], f32, tag="winv_bc")
            sfp_row = persist.tile([1, local_a], f32, tag="sfp_row")
            sw_row = persist.tile([1, wpad], f32, tag="sw_row")
            winv_row = persist.tile([1, wpad], f32, tag="winv_row")
            gsrt = persist.tile([128, wpad], bf16, tag="gsrt")
            g_am = persist.tile([128, wpad], bf16, tag="g_am")
            imgA = persist.tile([128, wpad], bf16, tag="imgA")
            imgB = persist.tile([128, wpad], bf16, tag="imgB")
            hs_pT = persist.tile([128, local_w], bf16, tag="hspT")
            w_bf = persist.tile([1, local_w], bf16, tag="wbf")
            comp_sum = persist.tile([128, 1], f32, tag="csum")
            ys_ar = persist.tile([128, 1], f32, tag="ysar")

            nc.sync.dma_start(sfp_row[:], t_sfp[:])
            nc.sync.dma_start(sw_row[:], t_sw[:])
            nc.sync.dma_start(winv_row[:], t_winv[:])

            # ---- collective DRAM buffers ----
            agh_in = []
            agh_out = []
            for l in range(LAYER_GNN):
                agh_in.append(dram.tile([128, local_a], bf16, tag=f"aghi{l}",
                                        name=f"aghi{l}"))
                agh_out.append(dram.tile([128 * ncores, local_a], bf16,
                                         tag=f"agho{l}", name=f"agho{l}",
                                         addr_space="Shared"))
            arc_in = dram.tile([128, 1], f32, tag="arci")
            arc_out = dram.tile([128, 1], f32, tag="arco", addr_space="Shared")
            arp_in = dram.tile([128, 1], f32, tag="arpi")
            arp_out = dram.tile([128, 1], f32, tag="arpo", addr_space="Shared")

            def bcast_row(row, dst, width):
                o = 0
                while o < width:
                    ms = min(512, width - o)
                    pb = ps4.tile([128, 512], f32, tag=f"b{(o // 512) % nblk_a}")
                    nc.tensor.matmul(pb[:, :ms], ones_f32[:], row[:, o:o + ms],
                                     start=True, stop=True)
                    nc.vector.tensor_copy(dst[:, o:o + ms], pb[:, :ms])
                    o += ms

            def onehot_gather(tab_ap, n_chunks, starts, W, bc, n_loc, nblk, dma_eng):
                """Accumulate one-hot gather into psum blocks; returns them."""
                pbs = []
                for b in range(nblk):
                    bw = min(512, n_loc - b * 512)
                    if b < nblk_a:
                        pt = ps4.tile([128, 512], f32, tag=f"b{b}")
                    else:
                        pt = ps_x.tile([128, 512], f32, tag="x")
                    pbs.append((pt, bw))
                    nc.tensor.matmul(pt[:, :bw], zcol[:], zrow[:, :bw],
                                     start=True, stop=False)
                for g in range(0, n_chunks, 8):
                    bs = min(8, n_chunks - g)
                    slab = tabp.tile([128, 8 * 128], bf16, tag="tab")
                    dma_eng.dma_start(slab[:, :bs * 128],
                                      tab_ap[:, g * 128:(g + bs) * 128])
                    for j in range(bs):
                        k = g + j
                        ck = starts[k]
                        we = min(W, n_loc - ck)
                        oh = ohp.tile([128, W_W], bf16, tag="oh")
                        nc.vector.tensor_scalar(
                            oh[:, :we], bc[:, ck:ck + we], float(128 * k),
                            iota_col[:], op0=SUB, op1=EQ)
                        for (b, lo, hi) in _segments(ck, W, n_loc):
                            nc.tensor.matmul(
                                pbs[b][0][:, lo - 512 * b:hi - 512 * b],
                                slab[:, j * 128:(j + 1) * 128],
                                oh[:, lo - ck:hi - ck],
                                start=False, stop=False)
                for (pt, bw) in pbs:
                    nc.tensor.matmul(pt[:, :bw], zcol[:], zrow[:, :bw],
                                     start=False, stop=True)
                return pbs

            def fp_gather():
                bcast_row(sfp_row, sfp_bc, local_a)
                pbs = onehot_gather(t_etab, nch_f, fstarts, W_F, sfp_bc,
                                    local_a, nblk_a, nc.scalar)
                for b, (pt, bw) in enumerate(pbs):
                    nc.vector.tensor_copy(xs[:, b * 512:b * 512 + bw], pt[:, :bw])

            def word_gather():
                bcast_row(sw_row, sw_bc, wpad)
                bcast_row(winv_row, winv_bc, wpad)
                pbs = onehot_gather(t_wtab, nch_w, wstarts, W_W, sw_bc,
                                    wpad, nblk_w, nc.scalar)
                for b, (pt, bw) in enumerate(pbs):
                    nc.vector.tensor_copy(gsrt[:, b * 512:b * 512 + bw], pt[:, :bw])
                # transpose sorted gather to position-major chunks
                for s in range(wch):
                    pt = ps_s.tile([128, 128], bf16, tag="s")
                    nc.tensor.transpose(pt[:], gsrt[:, s * 128:(s + 1) * 128],
                                        ident_bf[:])
                    nc.vector.tensor_copy(g_am[:, s * 128:(s + 1) * 128], pt[:])
                # unsort: img[d, j] = sum_s g_am[s, d] * (winv[j] == s)
                ipbs = []
                for b in range(nblk_w):
                    bw = min(512, wpad - b * 512)
                    if b < nblk_a:
                        pt = ps4.tile([128, 512], f32, tag=f"b{b}")
                    else:
                        pt = ps_x.tile([128, 512], f32, tag="x")
                    ipbs.append((pt, bw))
                for kc in range(wch):
                    ohu = ohup.tile([128, wpad], bf16, tag="ohu")
                    nc.vector.tensor_scalar(ohu[:], winv_bc[:], float(128 * kc),
                                            iota_col[:], op0=SUB, op1=EQ)
                    for b, (pt, bw) in enumerate(ipbs):
                        nc.tensor.matmul(pt[:, :bw],
                                         g_am[:, kc * 128:(kc + 1) * 128],
                                         ohu[:, b * 512:b * 512 + bw],
                                         start=(kc == 0), stop=(kc == wch - 1))
                for b, (pt, bw) in enumerate(ipbs):
                    nc.vector.tensor_copy(imgA[:, b * 512:b * 512 + bw], pt[:, :bw])

            bmat_r = t_bmat.rearrange("(t p) m -> t p m", p=128)
            ach = na // 128

            def gnn_hs_ag(layer):
                hs_am = hsamp.tile([128, local_a], bf16, tag="hsam")
                for o in range(local_a // 128):
                    hp = ps_s.tile([128, 128], f32, tag="s")
                    nc.tensor.matmul(hp[:], ones_f32[:], bg_row[:],
                                     start=True, stop=False)
                    nc.tensor.matmul(hp[:], xs[:, o * 128:(o + 1) * 128], wgT[:],
                                     start=False, stop=True)
                    nc.scalar.activation(hs_am[:, o * 128:(o + 1) * 128], hp[:], Relu)
                nc.sync.dma_start(agh_in[layer][:], hs_am[:])
                nc.gpsimd.collective_compute(
                    "AllGather", mybir.AluOpType.bypass,
                    ins=[agh_in[layer][:].opt()], outs=[agh_out[layer][:].opt()],
                    replica_groups=rg)
                for r in range(ncores):
                    nc.sync.dma_start(hs_all[r][:],
                                      agh_out[layer][r * 128:(r + 1) * 128, :])

            def gnn_spmm(layer):
                psums = [ps4.tile([128, 512], f32, tag=f"b{b}",
                                  name=f"spmm_l{layer}_{b}")
                         for b in range(nblk_a)]
                for ki in range(ach):
                    if ki % 2 == 0:
                        slab = slabp.tile([128, 2 * local_a], f8, tag="slab")
                        nc.sync.dma_start(slab[:, :local_a], bmat_r[ki])
                        nc.sync.dma_start(slab[:, local_a:], bmat_r[ki + 1])
                    part = slab[:, (ki % 2) * local_a:(ki % 2 + 1) * local_a]
                    lhsT = hs_all[ki // (local_a // 128)][
                        :, (ki % (local_a // 128)) * 128:(ki % (local_a // 128) + 1) * 128]
                    for b in range(nblk_a):
                        nc.tensor.matmul(psums[b][:], lhsT, part[:, b * 512:(b + 1) * 512],
                                         start=(ki == 0), stop=(ki == ach - 1))
                if layer < LAYER_GNN - 1:
                    for b in range(nblk_a):
                        nc.vector.tensor_tensor(xs[:, b * 512:(b + 1) * 512],
                                                xs[:, b * 512:(b + 1) * 512],
                                                psums[b][:], op=ADD)
                else:
                    # comp partial = sum_cols(xs2_local) + sum_cols(delta3_local)
                    acc = smallp.tile([128, 1], f32, tag="small")
                    tmp = smallp.tile([128, 1], f32, tag="small")
                    nc.vector.reduce_sum(acc[:], xs[:], axis=mybir.AxisListType.X)
                    for b in range(nblk_a):
                        nc.vector.reduce_sum(tmp[:], psums[b][:],
                                             axis=mybir.AxisListType.X)
                        nc.vector.tensor_tensor(acc[:], acc[:], tmp[:], op=ADD)
                    nc.sync.dma_start(arc_in[:], acc[:])
                    nc.gpsimd.collective_compute(
                        "AllReduce", mybir.AluOpType.add,
                        ins=[arc_in[:].opt()], outs=[arc_out[:].opt()],
                        replica_groups=rg)
                    nc.sync.dma_start(comp_sum[:], arc_out[:])

            def protein_conv():
                bufs = [imgA, imgB]
                for l in range(LAYER_CNN):
                    lo = WINDOW * (l + 1)
                    hi = lwin - WINDOW * (l + 1)
                    src, dst = bufs[l % 2], bufs[(l + 1) % 2]
                    o = lo
                    while o < hi:
                        ms = min(512, hi - o)
                        pc = ps_x.tile([128, 512], f32, tag="x")
                        for a in range(KK):
                            nc.tensor.matmul(
                                pc[:, :ms], convm[:, a * DIM:(a + 1) * DIM],
                                src[:, o + a - WINDOW:o + a - WINDOW + ms],
                                start=(a == 0), stop=(a == KK - 1))
                        nc.scalar.activation(dst[:, o:o + ms], pc[:, :ms], Relu,
                                             bias=convb_col[:])
                        o += ms
                    if l < LAYER_CNN - 1:
                        nc.vector.tensor_tensor(dst[:, lo:hi], dst[:, lo:hi],
                                                wmask_sb[:, lo:hi], op=MUL)

            def protein_hsp():
                xsp = [imgA, imgB][LAYER_CNN % 2]
                o = 0
                while o < local_w:
                    ms = min(512, local_w - o)
                    pc = ps_x.tile([128, 512], f32, tag="x")
                    nc.tensor.matmul(pc[:, :ms], waT[:], xsp[:, HALO + o:HALO + o + ms],
                                     start=True, stop=True)
                    nc.scalar.activation(hs_pT[:, o:o + ms], pc[:, :ms], Relu,
                                         bias=ba_col[:])
                    o += ms

            def tail():
                comp_bf = smallp.tile([128, 1], bf16, tag="smallbf")
                nc.vector.tensor_scalar_mul(comp_bf[:], comp_sum[:], 1.0 / na)
                ph = ps_s.tile([128, 128], f32, tag="s")
                nc.tensor.matmul(ph[:, :1], waT[:], comp_bf[:], start=True, stop=True)
                h_bf = smallp.tile([128, 1], bf16, tag="smallbf")
                nc.scalar.activation(h_bf[:], ph[:, :1], Relu, bias=ba_col[:])
                # w = tanh(h . hs_p)
                o = 0
                while o < local_w:
                    ms = min(512, local_w - o)
                    pw = ps_x.tile([128, 512], f32, tag="x")
                    nc.tensor.matmul(pw[:1, :ms], h_bf[:], hs_pT[:, o:o + ms],
                                     start=True, stop=True)
                    nc.scalar.activation(w_bf[:, o:o + ms], pw[:1, :ms], Tanh)
                    o += ms
                # ys partial = sum_l w[l] * hs_p[:, l]
                prev = None
                o = 0
                while o < local_w:
                    ms = min(512, local_w - o)
                    pb = ps_x.tile([128, 512], f32, tag="x")
                    nc.tensor.matmul(pb[:, :ms], ones_row[:], w_bf[:, o:o + ms],
                                     start=True, stop=True)
                    scr = scrp.tile([128, 512], f32, tag="scr")
                    nc.vector.tensor_tensor(scr[:, :ms], pb[:, :ms],
                                            hs_pT[:, o:o + ms], op=MUL)
                    acc = smallp.tile([128, 1], f32, tag="small")
                    nc.vector.reduce_sum(acc[:], scr[:, :ms], axis=mybir.AxisListType.X)
                    if prev is not None:
                        nc.vector.tensor_tensor(acc[:], acc[:], prev[:], op=ADD)
                    prev = acc
                    o += ms
                nc.sync.dma_start(arp_in[:], prev[:])
                nc.gpsimd.collective_compute(
                    "AllReduce", mybir.AluOpType.add,
                    ins=[arp_in[:].opt()], outs=[arp_out[:].opt()],
                    replica_groups=rg)
                nc.sync.dma_start(ys_ar[:], arp_out[:])
                # output MLP on cat = [compound, protein]
                cat = catp.tile([128, 2], f32, tag="cat")
                nc.vector.tensor_scalar_mul(cat[:, 0:1], comp_sum[:], 1.0 / na)
                nc.vector.tensor_scalar_mul(cat[:, 1:2], ys_ar[:], 1.0 / nw)
                for l in range(LAYER_OUT):
                    ncat = catp.tile([128, 2], f32, tag="cat")
                    for i in range(2):
                        pm = ps_s.tile([128, 128], f32, tag="s")
                        for j in range(2):
                            nc.tensor.matmul(pm[:, :1],
                                             woT_sb[:, (j * 2 + i) * DIM:(j * 2 + i + 1) * DIM],
                                             cat[:, j:j + 1],
                                             start=(j == 0), stop=(j == 1))
                        nc.scalar.activation(ncat[:, i:i + 1], pm[:, :1], Relu,
                                             bias=bo_sb[:, i:i + 1])
                    cat = ncat
                pf = ps_s.tile([128, 128], f32, tag="s")
                for j in range(2):
                    nc.tensor.matmul(pf[:2, :1], wiT_sb[:, 2 * j:2 * j + 2],
                                     cat[:, j:j + 1], start=(j == 0), stop=(j == 1))
                res = smallp.tile([2, 1], f32, tag="res")
                nc.scalar.activation(res[:], pf[:2, :1], Ident, bias=bi_sb[:])
                nc.sync.dma_start(t_out[:], res[:])

            def finish_early():
                res2 = smallp.tile([2, 1], f32, tag="res")
                nc.vector.tensor_copy(res2[:], bi_sb[:])
                nc.sync.dma_start(t_out[:], res2[:])

            if stage >= 1:
                fp_gather()
            if stage >= 3:
                gnn_hs_ag(0)
            if stage >= 2:
                word_gather()
            if stage >= 3:
                gnn_spmm(0)
            if stage >= 5:
                gnn_hs_ag(1)
            if stage >= 4:
                protein_conv()
            if stage >= 5:
                gnn_spmm(1)
            if stage >= 6:
                gnn_hs_ag(2)
            if stage >= 7:
                protein_hsp()
            if stage >= 6:
                gnn_spmm(2)
            if stage >= 7:
                tail()
            else:
                finish_early()

    nc.compile()
    return nc


def prep_in_maps(inputs, na, nw, nfp, nword, ncores):
    """Host-side sharding/layout prep (layout + lossless/near-lossless casts)."""
    local_a = na // ncores
    local_w = nw // ncores
    lwin = local_w + 2 * HALO
    wch = _ceil_div(lwin, 128)
    wpad = wch * 128
    tpad_f = _ceil_div(nfp, 128) * 128
    tpad_w = _ceil_div(nword + 1, 128) * 128
    fstarts = _win_starts(tpad_f // 128, local_a, nfp, W_F)
    wstarts = _win_starts(tpad_w // 128, wpad, nword, W_W)

    fingerprints = np.asarray(inputs["fingerprints"]).astype(np.int64)
    adjacency = np.asarray(inputs["adjacency"], dtype=np.float32)
    words = np.asarray(inputs["words"]).astype(np.int64)
    embed_fp = np.asarray(inputs["embed_fp"], dtype=np.float32)
    embed_word = np.asarray(inputs["embed_word"], dtype=np.float32)
    Wg = np.asarray(inputs["Wg"], dtype=np.float32)
    bg = np.asarray(inputs["bg"], dtype=np.float32)
    conv_k = np.asarray(inputs["conv_k"], dtype=np.float32)
    conv_b = np.asarray(inputs["conv_b"], dtype=np.float32)
    Wa = np.asarray(inputs["Wa"], dtype=np.float32)
    ba = np.asarray(inputs["ba"], dtype=np.float32)
    Wo = np.asarray(inputs["Wo"], dtype=np.float32)
    bo = np.asarray(inputs["bo"], dtype=np.float32)
    Wi = np.asarray(inputs["Wi"], dtype=np.float32)
    bi = np.asarray(inputs["bi"], dtype=np.float32)

    # ---- per-core fp sort; permutation absorbed into adjacency relayout ----
    orders = []
    perm = np.empty(na, np.int64)
    for c in range(ncores):
        sl = fingerprints[c * local_a:(c + 1) * local_a]
        o = np.argsort(sl, kind="stable")
        orders.append(o)
        perm[c * local_a:(c + 1) * local_a] = c * local_a + o

    # verify one-hot windows cover the sorted data
    for c in range(ncores):
        s = np.sort(fingerprints[c * local_a:(c + 1) * local_a])
        for k in range(tpad_f // 128):
            lo_i = np.searchsorted(s, 128 * k, "left")
            hi_i = np.searchsorted(s, min(128 * (k + 1), nfp), "left")
            if hi_i > lo_i:
                ck = fstarts[k]
                assert lo_i >= ck and hi_i <= ck + W_F, \
                    f"fp window overflow core {c} chunk {k}"

    # adjacency -> fp8 bit pattern, both axes permuted to sorted order
    a8 = (adjacency != 0).astype(np.uint8) * np.uint8(0x38)
    a8 = a8[perm][:, perm]

    # partition-major bf16 tables: tab_pm[p, t*128+d] = tab[t*128+p, d]
    def pm_table(tab, tpad):
        t = np.zeros((tpad, DIM), np.float32)
        t[:tab.shape[0]] = tab
        nch = tpad // 128
        return np.ascontiguousarray(
            t.reshape(nch, 128, DIM).transpose(1, 0, 2).reshape(128, nch * DIM)
        ).astype(BF16)

    etab_pm = pm_table(embed_fp, tpad_f)
    wtab_full = np.concatenate([embed_word, np.zeros((1, DIM), np.float32)], axis=0)
    wtab_pm = pm_table(wtab_full, tpad_w)

    K2 = conv_k[0, 0]
    M = np.zeros((DIM, KK * DIM), np.float32)
    for a in range(KK):
        Ma = np.zeros((DIM, DIM), np.float32)
        for b_ in range(KK):
            Ma += K2[a, b_] * np.eye(DIM, k=5 - b_, dtype=np.float32)
        M[:, a * DIM:(a + 1) * DIM] = Ma

    common = dict(
        etabpm=etab_pm,
        wtabpm=wtab_pm,
        iota_col=np.arange(128, dtype=np.float32).reshape(128, 1),
        wgT=np.ascontiguousarray(Wg.T).astype(np.float32),
        bg_row=bg.reshape(1, DIM).astype(np.float32),
        waT=np.ascontiguousarray(Wa.T).astype(BF16),
        ba_col=ba.reshape(DIM, 1).astype(np.float32),
        convm=M.astype(BF16),
        convb_col=np.full((DIM, 1), conv_b[0], np.float32),
        woT=np.ascontiguousarray(Wo.T).astype(np.float32),
        bo_col=bo.reshape(2 * DIM, 1).astype(np.float32),
        wiT=np.ascontiguousarray(Wi.T).astype(np.float32),
        bi_col=bi.reshape(2, 1).astype(np.float32),
        ones_row=np.ones((1, DIM), BF16),
    )

    in_maps = []
    for c in range(ncores):
        sl = slice(c * local_a, (c + 1) * local_a)
        bmat = np.ascontiguousarray(a8[sl, :].T).view(F8)
        sfp = np.sort(fingerprints[sl]).astype(np.float32).reshape(1, local_a)

        # word window: values (OOB/pad -> nword sentinel), sorted
        w0 = c * local_w - HALO
        pos = np.arange(wpad)
        gidx = w0 + pos
        valid = (gidx >= 0) & (gidx < nw) & (pos < lwin)
        widx = np.where(valid, words[np.clip(gidx, 0, nw - 1)], nword).astype(np.int64)
        order = np.argsort(widx, kind="stable")
        swidx = widx[order]
        # verify windows cover real words
        for k in range(tpad_w // 128):
            lo_i = np.searchsorted(swidx, 128 * k, "left")
            hi_i = np.searchsorted(swidx, min(128 * (k + 1), nword), "left")
            if hi_i > lo_i:
                ck = wstarts[k]
                assert lo_i >= ck and hi_i <= ck + W_W, \
                    f"word window overflow core {c} chunk {k}"
        # unsort index: winv[j] = sorted position of window position j
        # (OOB positions get an out-of-range index -> img column stays zero)
        winv = np.full(wpad, wpad + 128, np.float32)
        inv = np.empty(wpad, np.int64)
        inv[order] = np.arange(wpad)
        winv[valid] = inv[valid].astype(np.float32)

        wmask = np.broadcast_to(
            ((gidx >= 0) & (gidx < nw)).astype(BF16)[None, :], (128, wpad))
        m = dict(common)
        m.update(bmat=bmat,
                 sfp_row=sfp,
                 sw_row=swidx.astype(np.float32).reshape(1, wpad),
                 winv_row=winv.reshape(1, wpad),
                 wmask=np.ascontiguousarray(wmask))
        in_maps.append(m)
    return in_maps


_CACHE = {}


def _get_kernel(cfg_key):
    if cfg_key not in _CACHE:
        na, nw, nfp, nword, ncores = cfg_key
        _CACHE[cfg_key] = build_kernel(na, nw, nfp, nword, ncores)
    return _CACHE[cfg_key]


def kernel(**inputs) -> np.ndarray:
    from concourse import bass_utils
    cfg = FULL
    key = (cfg["na"], cfg["nw"], cfg["nfp"], cfg["nword"], cfg["ncores"])
    nc = _get_kernel(key)
    in_maps = prep_in_maps(inputs, *key)
    res = bass_utils.run_bass_kernel_spmd(
        nc, in_maps, core_ids=list(range(cfg["ncores"])), trace=False)
    out = np.asarray(res.results[0]["out"], np.float32).reshape(1, 2)
    return out


# revision 6
# speedup vs baseline: 20.1093x; 1.7036x over previous
"""CompoundProteinInteractionPrediction on 8 Trainium2 NeuronCores (Bass/Tile).

v2 design.  The v1 bottleneck was ~20ms of gpsimd indirect-DMA embedding
gathers (~1.1us per gathered row).  v2 replaces both gathers with one-hot
matmul gathers: the embedding table streams through SBUF contiguously as
matmul lhsT tiles while a one-hot rhs is built on-chip (DVE tensor_scalar
subtract+is_equal against an iota column).  Each core's atoms are sorted by
fingerprint on the host so the one-hot only needs a narrow fixed window of
output columns per 128-row table chunk; the atom permutation is absorbed
into the host-side relayout of the adjacency, so no on-device unsort is
needed for the GNN.  The protein word gather uses the same trick plus a
small on-device unsort matmul (conv needs sequence order).

GNN sharding: xs lives row-sharded (2048 atoms/core, fp-sorted order);
per layer each core computes its own hs chunk atom-major, AllGathers it
(bf16, 0.5MB), and runs the SpMM against its fp8 adjacency column block.
"""
import sys

sys.path.insert(0, "/opt/trn_rl_repo")

import numpy as np
import ml_dtypes

import concourse.bass as bass
import concourse.tile as tile
from concourse import bacc, mybir
from concourse.masks import make_identity

F8 = ml_dtypes.float8_e4m3
BF16 = ml_dtypes.bfloat16

DIM = 128
WINDOW = 5
KK = 2 * WINDOW + 1
LAYER_GNN = 3
LAYER_CNN = 3
LAYER_OUT = 2
HALO = WINDOW * LAYER_CNN  # 15

FULL = dict(na=16384, nw=16384, nfp=100000, nword=20000, ncores=8)

W_F = 192    # one-hot window width, fingerprint gather (max required: 128)
W_W = 384    # one-hot window width, word gather (max required: 320)


def _ceil_div(a, b):
    return (a + b - 1) // b


def _win_starts(n_chunks, n_loc, t_real, W):
    """Compile-time one-hot window start per 128-row table chunk."""
    out = []
    for k in range(n_chunks):
        exp = n_loc * (128 * k + 64) / t_real
        out.append(int(np.clip(round(exp - W / 2), 0, n_loc - W)))
    return out


def _segments(ck, W, n_loc):
    """Split window [ck, ck+W) at 512-col PSUM-bank boundaries."""
    segs = []
    lo = ck
    end = min(ck + W, n_loc)
    while lo < end:
        b = lo // 512
        hi = min(end, (b + 1) * 512)
        segs.append((b, lo, hi))
        lo = hi
    return segs


REPS = 8   # full-computation repetitions inside one NEFF (timing amortization)


def build_kernel(na, nw, nfp, nword, ncores, enable_asserts=False, stage=7, reps=1):
    local_a = na // ncores
    local_w = nw // ncores
    lwin = local_w + 2 * HALO      # gathered word window
    wch = _ceil_div(lwin, 128)
    wpad = wch * 128               # 2176
    tpad_f = _ceil_div(nfp, 128) * 128    # 100096
    tpad_w = _ceil_div(nword + 1, 128) * 128  # 20096
    nch_f = tpad_f // 128          # 782
    nch_w = tpad_w // 128          # 157
    fstarts = _win_starts(nch_f, local_a, nfp, W_F)
    wstarts = _win_starts(nch_w, wpad, nword, W_W)
    nblk_a = local_a // 512        # 4
    nblk_w = _ceil_div(wpad, 512)  # 5 (last block 128 wide)

    f32 = mybir.dt.float32
    bf16 = mybir.dt.bfloat16
    f8 = mybir.dt.float8e4
    Relu = mybir.ActivationFunctionType.Relu
    Tanh = mybir.ActivationFunctionType.Tanh
    Ident = mybir.ActivationFunctionType.Identity
    SUB = mybir.AluOpType.subtract
    EQ = mybir.AluOpType.is_equal
    ADD = mybir.AluOpType.add
    MUL = mybir.AluOpType.mult

    nc = bacc.Bacc("TRN2", target_bir_lowering=False, debug=False,
                   enable_asserts=enable_asserts, num_devices=ncores)

    # ---- DRAM parameters (per-core values supplied via in_maps) ----
    t_bmat = nc.dram_tensor("bmat", [na, local_a], f8, kind="ExternalInput").ap()
    t_etab = nc.dram_tensor("etabpm", [128, tpad_f], bf16, kind="ExternalInput").ap()
    t_wtab = nc.dram_tensor("wtabpm", [128, tpad_w], bf16, kind="ExternalInput").ap()
    t_sfp = nc.dram_tensor("sfp_row", [1, local_a], f32, kind="ExternalInput").ap()
    t_sw = nc.dram_tensor("sw_row", [1, wpad], f32, kind="ExternalInput").ap()
    t_winv = nc.dram_tensor("winv_row", [1, wpad], f32, kind="ExternalInput").ap()
    t_iota = nc.dram_tensor("iota_col", [128, 1], f32, kind="ExternalInput").ap()
    t_wgT = nc.dram_tensor("wgT", [DIM, DIM], f32, kind="ExternalInput").ap()
    t_bg = nc.dram_tensor("bg_row", [1, DIM], f32, kind="ExternalInput").ap()
    t_waT = nc.dram_tensor("waT", [DIM, DIM], bf16, kind="ExternalInput").ap()
    t_ba = nc.dram_tensor("ba_col", [DIM, 1], f32, kind="ExternalInput").ap()
    t_convm = nc.dram_tensor("convm", [DIM, KK * DIM], bf16, kind="ExternalInput").ap()
    t_convb = nc.dram_tensor("convb_col", [DIM, 1], f32, kind="ExternalInput").ap()
    t_woT = nc.dram_tensor("woT", [2 * DIM, 2 * DIM], f32, kind="ExternalInput").ap()
    t_bo = nc.dram_tensor("bo_col", [2 * DIM, 1], f32, kind="ExternalInput").ap()
    t_wiT = nc.dram_tensor("wiT", [2 * DIM, 2], f32, kind="ExternalInput").ap()
    t_bi = nc.dram_tensor("bi_col", [2, 1], f32, kind="ExternalInput").ap()
    t_ones = nc.dram_tensor("ones_row", [1, DIM], bf16, kind="ExternalInput").ap()
    t_wmask = nc.dram_tensor("wmask", [128, wpad], bf16, kind="ExternalInput").ap()
    t_out = nc.dram_tensor("out", [2, 1], f32, kind="ExternalOutput").ap()

    rg = [list(range(ncores))]

    with tile.TileContext(nc) as tc:
        with (
            tc.tile_pool(name="persist", bufs=1) as persist,
            tc.tile_pool(name="tabp", bufs=4) as tabp,
            tc.tile_pool(name="ohp", bufs=4) as ohp,
            tc.tile_pool(name="ohup", bufs=2) as ohup,
            tc.tile_pool(name="slabp", bufs=6) as slabp,
            tc.tile_pool(name="hsamp", bufs=2) as hsamp,
            tc.tile_pool(name="smallp", bufs=10) as smallp,
            tc.tile_pool(name="catp", bufs=3) as catp,
            tc.tile_pool(name="scrp", bufs=2) as scrp,
            tc.tile_pool(name="dram", bufs=1, space="DRAM") as dram,
            tc.tile_pool(name="ps4", bufs=1, space="PSUM") as ps4,
            tc.tile_pool(name="ps_x", bufs=2, space="PSUM") as ps_x,
            tc.tile_pool(name="ps_s", bufs=2, space="PSUM") as ps_s,
        ):
            # ---- small weights into SBUF ----
            wgT = persist.tile([DIM, DIM], f32, tag="wgT")
            bg_row = persist.tile([1, DIM], f32, tag="bg_row")
            waT = persist.tile([DIM, DIM], bf16, tag="waT")
            ba_col = persist.tile([DIM, 1], f32, tag="ba_col")
            convm = persist.tile([DIM, KK * DIM], bf16, tag="convm")
            convb_col = persist.tile([DIM, 1], f32, tag="convb_col")
            woT_sb = persist.tile([DIM, 4 * DIM], f32, tag="woT_sb")
            bo_sb = persist.tile([DIM, 2], f32, tag="bo_sb")
            wiT_sb = persist.tile([DIM, 4], f32, tag="wiT_sb")
            bi_sb = persist.tile([2, 1], f32, tag="bi_sb")
            ones_row = persist.tile([1, DIM], bf16, tag="ones_row")
            iota_col = persist.tile([128, 1], f32, tag="iota_col")
            wmask_sb = persist.tile([128, wpad], bf16, tag="wmask_sb")
            ident_bf = persist.tile([DIM, DIM], bf16, tag="ident_bf")
            ones_f32 = persist.tile([1, DIM], f32, tag="ones_f32")
            zcol = persist.tile([1, DIM], bf16, tag="zcol")
            zrow = persist.tile([1, 512], bf16, tag="zrow")

            nc.sync.dma_start(wgT[:], t_wgT[:])
            nc.sync.dma_start(bg_row[:], t_bg[:])
            nc.sync.dma_start(waT[:], t_waT[:])
            nc.sync.dma_start(ba_col[:], t_ba[:])
            nc.sync.dma_start(convm[:], t_convm[:])
            nc.sync.dma_start(convb_col[:], t_convb[:])
            for j in range(2):
                for i in range(2):
                    nc.sync.dma_start(
                        woT_sb[:, (j * 2 + i) * DIM:(j * 2 + i + 1) * DIM],
                        t_woT[j * DIM:(j + 1) * DIM, i * DIM:(i + 1) * DIM])
                nc.sync.dma_start(bo_sb[:, j:j + 1], t_bo[j * DIM:(j + 1) * DIM, :])
                nc.sync.dma_start(wiT_sb[:, 2 * j:2 * j + 2], t_wiT[j * DIM:(j + 1) * DIM, :])
            nc.sync.dma_start(bi_sb[:], t_bi[:])
            nc.sync.dma_start(ones_row[:], t_ones[:])
            nc.sync.dma_start(iota_col[:], t_iota[:])
            nc.sync.dma_start(wmask_sb[:], t_wmask[:])
            make_identity(nc, ident_bf[:])
            nc.gpsimd.memset(ones_f32[:], 1.0)
            nc.gpsimd.memset(zcol[:], 0.0)
            nc.gpsimd.memset(zrow[:], 0.0)

            # ---- persistent state ----
            xs = persist.tile([128, local_a], f32, tag="xs")           # dim-major
            hs_all = [persist.tile([128, local_a], bf16, tag=f"hsall{r}",
                                   name=f"hsall{r}")
                      for r in range(ncores)]
            sfp_bc = persist.tile([128, local_a], f32, tag="sfp_bc")
            sw_bc = persist.tile([128, wpad], f32, tag="sw_bc")
            winv_bc = persist.tile([128, wpad], f32, tag="winv_bc")
            sfp_row = persist.tile([1, local_a], f32, tag="sfp_row")
            sw_row = persist.tile([1, wpad], f32, tag="sw_row")
            winv_row = persist.tile([1, wpad], f32, tag="winv_row")
            gsrt = persist.tile([128, wpad], bf16, tag="gsrt")
            g_am = persist.tile([128, wpad], bf16, tag="g_am")
            imgA = persist.tile([128, wpad], bf16, tag="imgA")
            imgB = persist.tile([128, wpad], bf16, tag="imgB")
            hs_pT = persist.tile([128, local_w], bf16, tag="hspT")
            w_bf = persist.tile([1, local_w], bf16, tag="wbf")
            comp_sum = persist.tile([128, 1], f32, tag="csum")
            ys_ar = persist.tile([128, 1], f32, tag="ysar")

            nc.sync.dma_start(sfp_row[:], t_sfp[:])
            nc.sync.dma_start(sw_row[:], t_sw[:])
            nc.sync.dma_start(winv_row[:], t_winv[:])

            # ---- collective DRAM buffers (fresh set per rep) ----
            comm = {}

            def alloc_comm(rep):
                agh_in, agh_out = [], []
                for l in range(LAYER_GNN):
                    agh_in.append(dram.tile([128, local_a], bf16,
                                            tag=f"aghi{rep}_{l}",
                                            name=f"aghi{rep}_{l}"))
                    agh_out.append(dram.tile([128 * ncores, local_a], bf16,
                                             tag=f"agho{rep}_{l}",
                                             name=f"agho{rep}_{l}",
                                             addr_space="Shared"))
                comm["agh_in"] = agh_in
                comm["agh_out"] = agh_out
                comm["arc_in"] = dram.tile([128, 1], f32, tag=f"arci{rep}",
                                           name=f"arci{rep}")
                comm["arc_out"] = dram.tile([128, 1], f32, tag=f"arco{rep}",
                                            name=f"arco{rep}", addr_space="Shared")
                comm["arp_in"] = dram.tile([128, 1], f32, tag=f"arpi{rep}",
                                           name=f"arpi{rep}")
                comm["arp_out"] = dram.tile([128, 1], f32, tag=f"arpo{rep}",
                                            name=f"arpo{rep}", addr_space="Shared")

            def bcast_row(row, dst, width):
                o = 0
                while o < width:
                    ms = min(512, width - o)
                    pb = ps4.tile([128, 512], f32, tag=f"b{(o // 512) % nblk_a}")
                    nc.tensor.matmul(pb[:, :ms], ones_f32[:], row[:, o:o + ms],
                                     start=True, stop=True)
                    nc.vector.tensor_copy(dst[:, o:o + ms], pb[:, :ms])
                    o += ms

            def onehot_gather(tab_ap, n_chunks, starts, W, bc, n_loc, nblk, dma_eng):
                """Accumulate one-hot gather into psum blocks; returns them."""
                pbs = []
                for b in range(nblk):
                    bw = min(512, n_loc - b * 512)
                    if b < nblk_a:
                        pt = ps4.tile([128, 512], f32, tag=f"b{b}")
                    else:
                        pt = ps_x.tile([128, 512], f32, tag="x")
                    pbs.append((pt, bw))
                    nc.tensor.matmul(pt[:, :bw], zcol[:], zrow[:, :bw],
                                     start=True, stop=False)
                for g in range(0, n_chunks, 8):
                    bs = min(8, n_chunks - g)
                    slab = tabp.tile([128, 8 * 128], bf16, tag="tab")
                    dma_eng.dma_start(slab[:, :bs * 128],
                                      tab_ap[:, g * 128:(g + bs) * 128])
                    for j in range(bs):
                        k = g + j
                        ck = starts[k]
                        we = min(W, n_loc - ck)
                        oh = ohp.tile([128, W_W], bf16, tag="oh")
                        nc.vector.tensor_scalar(
                            oh[:, :we], bc[:, ck:ck + we], float(128 * k),
                            iota_col[:], op0=SUB, op1=EQ)
                        for (b, lo, hi) in _segments(ck, W, n_loc):
                            nc.tensor.matmul(
                                pbs[b][0][:, lo - 512 * b:hi - 512 * b],
                                slab[:, j * 128:(j + 1) * 128],
                                oh[:, lo - ck:hi - ck],
                                start=False, stop=False)
                for (pt, bw) in pbs:
                    nc.tensor.matmul(pt[:, :bw], zcol[:], zrow[:, :bw],
                                     start=False, stop=True)
                return pbs

            def fp_gather():
                bcast_row(sfp_row, sfp_bc, local_a)
                pbs = onehot_gather(t_etab, nch_f, fstarts, W_F, sfp_bc,
                                    local_a, nblk_a, nc.scalar)
                for b, (pt, bw) in enumerate(pbs):
                    nc.vector.tensor_copy(xs[:, b * 512:b * 512 + bw], pt[:, :bw])

            def word_gather():
                bcast_row(sw_row, sw_bc, wpad)
                bcast_row(winv_row, winv_bc, wpad)
                pbs = onehot_gather(t_wtab, nch_w, wstarts, W_W, sw_bc,
                                    wpad, nblk_w, nc.scalar)
                for b, (pt, bw) in enumerate(pbs):
                    nc.vector.tensor_copy(gsrt[:, b * 512:b * 512 + bw], pt[:, :bw])
                # transpose sorted gather to position-major chunks
                for s in range(wch):
                    pt = ps_s.tile([128, 128], bf16, tag="s")
                    nc.tensor.transpose(pt[:], gsrt[:, s * 128:(s + 1) * 128],
                                        ident_bf[:])
                    nc.vector.tensor_copy(g_am[:, s * 128:(s + 1) * 128], pt[:])
                # unsort: img[d, j] = sum_s g_am[s, d] * (winv[j] == s)
                ipbs = []
                for b in range(nblk_w):
                    bw = min(512, wpad - b * 512)
                    if b < nblk_a:
                        pt = ps4.tile([128, 512], f32, tag=f"b{b}")
                    else:
                        pt = ps_x.tile([128, 512], f32, tag="x")
                    ipbs.append((pt, bw))
                for kc in range(wch):
                    ohu = ohup.tile([128, wpad], bf16, tag="ohu")
                    nc.vector.tensor_scalar(ohu[:], winv_bc[:], float(128 * kc),
                                            iota_col[:], op0=SUB, op1=EQ)
                    for b, (pt, bw) in enumerate(ipbs):
                        nc.tensor.matmul(pt[:, :bw],
                                         g_am[:, kc * 128:(kc + 1) * 128],
                                         ohu[:, b * 512:b * 512 + bw],
                                         start=(kc == 0), stop=(kc == wch - 1))
                for b, (pt, bw) in enumerate(ipbs):
                    nc.vector.tensor_copy(imgA[:, b * 512:b * 512 + bw], pt[:, :bw])

            bmat_r = t_bmat.rearrange("(t p) m -> t p m", p=128)
            ach = na // 128

            def gnn_hs_ag(layer):
                hs_am = hsamp.tile([128, local_a], bf16, tag="hsam")
                for o in range(local_a // 128):
                    hp = ps_s.tile([128, 128], f32, tag="s")
                    nc.tensor.matmul(hp[:], ones_f32[:], bg_row[:],
                                     start=True, stop=False)
                    nc.tensor.matmul(hp[:], xs[:, o * 128:(o + 1) * 128], wgT[:],
                                     start=False, stop=True)
                    nc.scalar.activation(hs_am[:, o * 128:(o + 1) * 128], hp[:], Relu)
                nc.sync.dma_start(comm["agh_in"][layer][:], hs_am[:])
                nc.gpsimd.collective_compute(
                    "AllGather", mybir.AluOpType.bypass,
                    ins=[comm["agh_in"][layer][:].opt()],
                    outs=[comm["agh_out"][layer][:].opt()],
                    replica_groups=rg)
                for r in range(ncores):
                    nc.sync.dma_start(hs_all[r][:],
                                      comm["agh_out"][layer][r * 128:(r + 1) * 128, :])

            def gnn_spmm(layer):
                psums = [ps4.tile([128, 512], f32, tag=f"b{b}",
                                  name=f"spmm_l{layer}_{b}")
                         for b in range(nblk_a)]
                for ki in range(ach):
                    if ki % 2 == 0:
                        slab = slabp.tile([128, 2 * local_a], f8, tag="slab")
                        nc.sync.dma_start(slab[:, :local_a], bmat_r[ki])
                        nc.sync.dma_start(slab[:, local_a:], bmat_r[ki + 1])
                    part = slab[:, (ki % 2) * local_a:(ki % 2 + 1) * local_a]
                    lhsT = hs_all[ki // (local_a // 128)][
                        :, (ki % (local_a // 128)) * 128:(ki % (local_a // 128) + 1) * 128]
                    for b in range(nblk_a):
                        nc.tensor.matmul(psums[b][:], lhsT, part[:, b * 512:(b + 1) * 512],
                                         start=(ki == 0), stop=(ki == ach - 1))
                if layer < LAYER_GNN - 1:
                    for b in range(nblk_a):
                        nc.vector.tensor_tensor(xs[:, b * 512:(b + 1) * 512],
                                                xs[:, b * 512:(b + 1) * 512],
                                                psums[b][:], op=ADD)
                else:
                    # comp partial = sum_cols(xs2_local) + sum_cols(delta3_local)
                    acc = smallp.tile([128, 1], f32, tag="small")
                    tmp = smallp.tile([128, 1], f32, tag="small")
                    nc.vector.reduce_sum(acc[:], xs[:], axis=mybir.AxisListType.X)
                    for b in range(nblk_a):
                        nc.vector.reduce_sum(tmp[:], psums[b][:],
                                             axis=mybir.AxisListType.X)
                        nc.vector.tensor_tensor(acc[:], acc[:], tmp[:], op=ADD)
                    nc.sync.dma_start(comm["arc_in"][:], acc[:])
                    nc.gpsimd.collective_compute(
                        "AllReduce", mybir.AluOpType.add,
                        ins=[comm["arc_in"][:].opt()], outs=[comm["arc_out"][:].opt()],
                        replica_groups=rg)
                    nc.sync.dma_start(comp_sum[:], comm["arc_out"][:])

            def protein_conv():
                bufs = [imgA, imgB]
                for l in range(LAYER_CNN):
                    lo = WINDOW * (l + 1)
                    hi = lwin - WINDOW * (l + 1)
                    src, dst = bufs[l % 2], bufs[(l + 1) % 2]
                    o = lo
                    while o < hi:
                        ms = min(512, hi - o)
                        pc = ps_x.tile([128, 512], f32, tag="x")
                        for a in range(KK):
                            nc.tensor.matmul(
                                pc[:, :ms], convm[:, a * DIM:(a + 1) * DIM],
                                src[:, o + a - WINDOW:o + a - WINDOW + ms],
                                start=(a == 0), stop=(a == KK - 1))
                        nc.scalar.activation(dst[:, o:o + ms], pc[:, :ms], Relu,
                                             bias=convb_col[:])
                        o += ms
                    if l < LAYER_CNN - 1:
                        nc.vector.tensor_tensor(dst[:, lo:hi], dst[:, lo:hi],
                                                wmask_sb[:, lo:hi], op=MUL)

            def protein_hsp():
                xsp = [imgA, imgB][LAYER_CNN % 2]
                o = 0
                while o < local_w:
                    ms = min(512, local_w - o)
                    pc = ps_x.tile([128, 512], f32, tag="x")
                    nc.tensor.matmul(pc[:, :ms], waT[:], xsp[:, HALO + o:HALO + o + ms],
                                     start=True, stop=True)
                    nc.scalar.activation(hs_pT[:, o:o + ms], pc[:, :ms], Relu,
                                         bias=ba_col[:])
                    o += ms

            def tail():
                comp_bf = smallp.tile([128, 1], bf16, tag="smallbf")
                nc.vector.tensor_scalar_mul(comp_bf[:], comp_sum[:], 1.0 / na)
                ph = ps_s.tile([128, 128], f32, tag="s")
                nc.tensor.matmul(ph[:, :1], waT[:], comp_bf[:], start=True, stop=True)
                h_bf = smallp.tile([128, 1], bf16, tag="smallbf")
                nc.scalar.activation(h_bf[:], ph[:, :1], Relu, bias=ba_col[:])
                # w = tanh(h . hs_p)
                o = 0
                while o < local_w:
                    ms = min(512, local_w - o)
                    pw = ps_x.tile([128, 512], f32, tag="x")
                    nc.tensor.matmul(pw[:1, :ms], h_bf[:], hs_pT[:, o:o + ms],
                                     start=True, stop=True)
                    nc.scalar.activation(w_bf[:, o:o + ms], pw[:1, :ms], Tanh)
                    o += ms
                # ys partial = sum_l w[l] * hs_p[:, l]
                prev = None
                o = 0
                while o < local_w:
                    ms = min(512, local_w - o)
                    pb = ps_x.tile([128, 512], f32, tag="x")
                    nc.tensor.matmul(pb[:, :ms], ones_row[:], w_bf[:, o:o + ms],
                                     start=True, stop=True)
                    scr = scrp.tile([128, 512], f32, tag="scr")
                    nc.vector.tensor_tensor(scr[:, :ms], pb[:, :ms],
                                            hs_pT[:, o:o + ms], op=MUL)
                    acc = smallp.tile([128, 1], f32, tag="small")
                    nc.vector.reduce_sum(acc[:], scr[:, :ms], axis=mybir.AxisListType.X)
                    if prev is not None:
                        nc.vector.tensor_tensor(acc[:], acc[:], prev[:], op=ADD)
                    prev = acc
                    o += ms
                nc.sync.dma_start(comm["arp_in"][:], prev[:])
                nc.gpsimd.collective_compute(
                    "AllReduce", mybir.AluOpType.add,
                    ins=[comm["arp_in"][:].opt()], outs=[comm["arp_out"][:].opt()],
                    replica_groups=rg)
                nc.sync.dma_start(ys_ar[:], comm["arp_out"][:])
                # output MLP on cat = [compound, protein]
                cat = catp.tile([128, 2], f32, tag="cat")
                nc.vector.tensor_scalar_mul(cat[:, 0:1], comp_sum[:], 1.0 / na)
                nc.vector.tensor_scalar_mul(cat[:, 1:2], ys_ar[:], 1.0 / nw)
                for l in range(LAYER_OUT):
                    ncat = catp.tile([128, 2], f32, tag="cat")
                    for i in range(2):
                        pm = ps_s.tile([128, 128], f32, tag="s")
                        for j in range(2):
                            nc.tensor.matmul(pm[:, :1],
                                             woT_sb[:, (j * 2 + i) * DIM:(j * 2 + i + 1) * DIM],
                                             cat[:, j:j + 1],
                                             start=(j == 0), stop=(j == 1))
                        nc.scalar.activation(ncat[:, i:i + 1], pm[:, :1], Relu,
                                             bias=bo_sb[:, i:i + 1])
                    cat = ncat
                pf = ps_s.tile([128, 128], f32, tag="s")
                for j in range(2):
                    nc.tensor.matmul(pf[:2, :1], wiT_sb[:, 2 * j:2 * j + 2],
                                     cat[:, j:j + 1], start=(j == 0), stop=(j == 1))
                res = smallp.tile([2, 1], f32, tag="res")
                nc.scalar.activation(res[:], pf[:2, :1], Ident, bias=bi_sb[:])
                nc.sync.dma_start(t_out[:], res[:])

            def finish_early():
                res2 = smallp.tile([2, 1], f32, tag="res")
                nc.vector.tensor_copy(res2[:], bi_sb[:])
                nc.sync.dma_start(t_out[:], res2[:])

            for rep in range(reps):
                alloc_comm(rep)
                if stage >= 1:
                    fp_gather()
                if stage >= 3:
                    gnn_hs_ag(0)
                if stage >= 2:
                    word_gather()
                if stage >= 3:
                    gnn_spmm(0)
                if stage >= 5:
                    gnn_hs_ag(1)
                if stage >= 4:
                    protein_conv()
                if stage >= 5:
                    gnn_spmm(1)
                if stage >= 6:
                    gnn_hs_ag(2)
                if stage >= 7:
                    protein_hsp()
                if stage >= 6:
                    gnn_spmm(2)
                if stage >= 7:
                    tail()
                else:
                    finish_early()

    nc.compile()
    return nc


def prep_in_maps(inputs, na, nw, nfp, nword, ncores):
    """Host-side sharding/layout prep (layout + lossless/near-lossless casts)."""
    local_a = na // ncores
    local_w = nw // ncores
    lwin = local_w + 2 * HALO
    wch = _ceil_div(lwin, 128)
    wpad = wch * 128
    tpad_f = _ceil_div(nfp, 128) * 128
    tpad_w = _ceil_div(nword + 1, 128) * 128
    fstarts = _win_starts(tpad_f // 128, local_a, nfp, W_F)
    wstarts = _win_starts(tpad_w // 128, wpad, nword, W_W)

    fingerprints = np.asarray(inputs["fingerprints"]).astype(np.int64)
    adjacency = np.asarray(inputs["adjacency"], dtype=np.float32)
    words = np.asarray(inputs["words"]).astype(np.int64)
    embed_fp = np.asarray(inputs["embed_fp"], dtype=np.float32)
    embed_word = np.asarray(inputs["embed_word"], dtype=np.float32)
    Wg = np.asarray(inputs["Wg"], dtype=np.float32)
    bg = np.asarray(inputs["bg"], dtype=np.float32)
    conv_k = np.asarray(inputs["conv_k"], dtype=np.float32)
    conv_b = np.asarray(inputs["conv_b"], dtype=np.float32)
    Wa = np.asarray(inputs["Wa"], dtype=np.float32)
    ba = np.asarray(inputs["ba"], dtype=np.float32)
    Wo = np.asarray(inputs["Wo"], dtype=np.float32)
    bo = np.asarray(inputs["bo"], dtype=np.float32)
    Wi = np.asarray(inputs["Wi"], dtype=np.float32)
    bi = np.asarray(inputs["bi"], dtype=np.float32)

    # ---- per-core fp sort; permutation absorbed into adjacency relayout ----
    orders = []
    perm = np.empty(na, np.int64)
    for c in range(ncores):
        sl = fingerprints[c * local_a:(c + 1) * local_a]
        o = np.argsort(sl, kind="stable")
        orders.append(o)
        perm[c * local_a:(c + 1) * local_a] = c * local_a + o

    # verify one-hot windows cover the sorted data
    for c in range(ncores):
        s = np.sort(fingerprints[c * local_a:(c + 1) * local_a])
        for k in range(tpad_f // 128):
            lo_i = np.searchsorted(s, 128 * k, "left")
            hi_i = np.searchsorted(s, min(128 * (k + 1), nfp), "left")
            if hi_i > lo_i:
                ck = fstarts[k]
                assert lo_i >= ck and hi_i <= ck + W_F, \
                    f"fp window overflow core {c} chunk {k}"

    # adjacency -> fp8 bit pattern, both axes permuted to sorted order
    a8 = (adjacency != 0).astype(np.uint8) * np.uint8(0x38)
    a8 = a8[perm][:, perm]

    # partition-major bf16 tables: tab_pm[p, t*128+d] = tab[t*128+p, d]
    def pm_table(tab, tpad):
        t = np.zeros((tpad, DIM), np.float32)
        t[:tab.shape[0]] = tab
        nch = tpad // 128
        return np.ascontiguousarray(
            t.reshape(nch, 128, DIM).transpose(1, 0, 2).reshape(128, nch * DIM)
        ).astype(BF16)

    etab_pm = pm_table(embed_fp, tpad_f)
    wtab_full = np.concatenate([embed_word, np.zeros((1, DIM), np.float32)], axis=0)
    wtab_pm = pm_table(wtab_full, tpad_w)

    K2 = conv_k[0, 0]
    M = np.zeros((DIM, KK * DIM), np.float32)
    for a in range(KK):
        Ma = np.zeros((DIM, DIM), np.float32)
        for b_ in range(KK):
            Ma += K2[a, b_] * np.eye(DIM, k=5 - b_, dtype=np.float32)
        M[:, a * DIM:(a + 1) * DIM] = Ma

    common = dict(
        etabpm=etab_pm,
        wtabpm=wtab_pm,
        iota_col=np.arange(128, dtype=np.float32).reshape(128, 1),
        wgT=np.ascontiguousarray(Wg.T).astype(np.float32),
        bg_row=bg.reshape(1, DIM).astype(np.float32),
        waT=np.ascontiguousarray(Wa.T).astype(BF16),
        ba_col=ba.reshape(DIM, 1).astype(np.float32),
        convm=M.astype(BF16),
        convb_col=np.full((DIM, 1), conv_b[0], np.float32),
        woT=np.ascontiguousarray(Wo.T).astype(np.float32),
        bo_col=bo.reshape(2 * DIM, 1).astype(np.float32),
        wiT=np.ascontiguousarray(Wi.T).astype(np.float32),
        bi_col=bi.reshape(2, 1).astype(np.float32),
        ones_row=np.ones((1, DIM), BF16),
    )

    in_maps = []
    for c in range(ncores):
        sl = slice(c * local_a, (c + 1) * local_a)
        bmat = np.ascontiguousarray(a8[sl, :].T).view(F8)
        sfp = np.sort(fingerprints[sl]).astype(np.float32).reshape(1, local_a)

        # word window: values (OOB/pad -> nword sentinel), sorted
        w0 = c * local_w - HALO
        pos = np.arange(wpad)
        gidx = w0 + pos
        valid = (gidx >= 0) & (gidx < nw) & (pos < lwin)
        widx = np.where(valid, words[np.clip(gidx, 0, nw - 1)], nword).astype(np.int64)
        order = np.argsort(widx, kind="stable")
        swidx = widx[order]
        # verify windows cover real words
        for k in range(tpad_w // 128):
            lo_i = np.searchsorted(swidx, 128 * k, "left")
            hi_i = np.searchsorted(swidx, min(128 * (k + 1), nword), "left")
            if hi_i > lo_i:
                ck = wstarts[k]
                assert lo_i >= ck and hi_i <= ck + W_W, \
                    f"word window overflow core {c} chunk {k}"
        # unsort index: winv[j] = sorted position of window position j
        # (OOB positions get an out-of-range index -> img column stays zero)
        winv = np.full(wpad, wpad + 128, np.float32)
        inv = np.empty(wpad, np.int64)
        inv[order] = np.arange(wpad)
        winv[valid] = inv[valid].astype(np.float32)

        wmask = np.broadcast_to(
            ((gidx >= 0) & (gidx < nw)).astype(BF16)[None, :], (128, wpad))
        m = dict(common)
        m.update(bmat=bmat,
                 sfp_row=sfp,
                 sw_row=swidx.astype(np.float32).reshape(1, wpad),
                 winv_row=winv.reshape(1, wpad),
                 wmask=np.ascontiguousarray(wmask))
        in_maps.append(m)
    return in_maps


_CACHE = {}


def _get_kernel(cfg_key):
    if cfg_key not in _CACHE:
        na, nw, nfp, nword, ncores = cfg_key
        _CACHE[cfg_key] = build_kernel(na, nw, nfp, nword, ncores)
    return _CACHE[cfg_key]


def kernel(**inputs) -> np.ndarray:
    from concourse import bass_utils
    cfg = FULL
    key = (cfg["na"], cfg["nw"], cfg["nfp"], cfg["nword"], cfg["ncores"])
    nc = _get_kernel(key)
    in_maps = prep_in_maps(inputs, *key)
    res = bass_utils.run_bass_kernel_spmd(
        nc, in_maps, core_ids=list(range(cfg["ncores"])), trace=False)
    out = np.asarray(res.results[0]["out"], np.float32).reshape(1, 2)
    return out


# revision 7
# speedup vs baseline: 23.2136x; 1.1544x over previous
"""CompoundProteinInteractionPrediction on 8 Trainium2 NeuronCores (Bass/Tile).

v2 design.  The v1 bottleneck was ~20ms of gpsimd indirect-DMA embedding
gathers (~1.1us per gathered row).  v2 replaces both gathers with one-hot
matmul gathers: the embedding table streams through SBUF contiguously as
matmul lhsT tiles while a one-hot rhs is built on-chip (DVE tensor_scalar
subtract+is_equal against an iota column).  Each core's atoms are sorted by
fingerprint on the host so the one-hot only needs a narrow fixed window of
output columns per 128-row table chunk; the atom permutation is absorbed
into the host-side relayout of the adjacency, so no on-device unsort is
needed for the GNN.  The protein word gather uses the same trick plus a
small on-device unsort matmul (conv needs sequence order).

GNN sharding: xs lives row-sharded (2048 atoms/core, fp-sorted order);
per layer each core computes its own hs chunk atom-major, AllGathers it
(bf16, 0.5MB), and runs the SpMM against its fp8 adjacency column block.
"""
import sys

sys.path.insert(0, "/opt/trn_rl_repo")

import numpy as np
import ml_dtypes

import concourse.bass as bass
import concourse.tile as tile
from concourse import bacc, mybir
from concourse.masks import make_identity

F8 = ml_dtypes.float8_e4m3
BF16 = ml_dtypes.bfloat16

DIM = 128
WINDOW = 5
KK = 2 * WINDOW + 1
LAYER_GNN = 3
LAYER_CNN = 3
LAYER_OUT = 2
HALO = WINDOW * LAYER_CNN  # 15

FULL = dict(na=16384, nw=16384, nfp=100000, nword=20000, ncores=8)

W_F = 192    # one-hot window width, fingerprint gather (max required: 128)
W_W = 384    # one-hot window width, word gather (max required: 320)


def _ceil_div(a, b):
    return (a + b - 1) // b


def _win_starts(n_chunks, n_loc, t_real, W):
    """Compile-time one-hot window start per 128-row table chunk."""
    out = []
    for k in range(n_chunks):
        exp = n_loc * (128 * k + 64) / t_real
        out.append(int(np.clip(round(exp - W / 2), 0, n_loc - W)))
    return out


def _segments(ck, W, n_loc):
    """Split window [ck, ck+W) at 512-col PSUM-bank boundaries."""
    segs = []
    lo = ck
    end = min(ck + W, n_loc)
    while lo < end:
        b = lo // 512
        hi = min(end, (b + 1) * 512)
        segs.append((b, lo, hi))
        lo = hi
    return segs


REPS = 16  # full-computation repetitions inside one NEFF (timing amortization)


def build_kernel(na, nw, nfp, nword, ncores, enable_asserts=False, stage=7, reps=1):
    local_a = na // ncores
    local_w = nw // ncores
    lwin = local_w + 2 * HALO      # gathered word window
    wch = _ceil_div(lwin, 128)
    wpad = wch * 128               # 2176
    tpad_f = _ceil_div(nfp, 128) * 128    # 100096
    tpad_w = _ceil_div(nword + 1, 128) * 128  # 20096
    nch_f = tpad_f // 128          # 782
    nch_w = tpad_w // 128          # 157
    fstarts = _win_starts(nch_f, local_a, nfp, W_F)
    wstarts = _win_starts(nch_w, wpad, nword, W_W)
    nblk_a = local_a // 512        # 4
    nblk_w = _ceil_div(wpad, 512)  # 5 (last block 128 wide)

    f32 = mybir.dt.float32
    bf16 = mybir.dt.bfloat16
    f8 = mybir.dt.float8e4
    Relu = mybir.ActivationFunctionType.Relu
    Tanh = mybir.ActivationFunctionType.Tanh
    Ident = mybir.ActivationFunctionType.Identity
    SUB = mybir.AluOpType.subtract
    EQ = mybir.AluOpType.is_equal
    ADD = mybir.AluOpType.add
    MUL = mybir.AluOpType.mult

    nc = bacc.Bacc("TRN2", target_bir_lowering=False, debug=False,
                   enable_asserts=enable_asserts, num_devices=ncores)

    # ---- DRAM parameters (per-core values supplied via in_maps) ----
    t_bmat = nc.dram_tensor("bmat", [na, local_a], f8, kind="ExternalInput").ap()
    t_etab = nc.dram_tensor("etabpm", [128, tpad_f], bf16, kind="ExternalInput").ap()
    t_wtab = nc.dram_tensor("wtabpm", [128, tpad_w], bf16, kind="ExternalInput").ap()
    t_sfp = nc.dram_tensor("sfp_row", [1, local_a], f32, kind="ExternalInput").ap()
    t_sw = nc.dram_tensor("sw_row", [1, wpad], f32, kind="ExternalInput").ap()
    t_winv = nc.dram_tensor("winv_row", [1, wpad], f32, kind="ExternalInput").ap()
    t_iota = nc.dram_tensor("iota_col", [128, 1], f32, kind="ExternalInput").ap()
    t_wgT = nc.dram_tensor("wgT", [DIM, DIM], f32, kind="ExternalInput").ap()
    t_bg = nc.dram_tensor("bg_row", [1, DIM], f32, kind="ExternalInput").ap()
    t_waT = nc.dram_tensor("waT", [DIM, DIM], bf16, kind="ExternalInput").ap()
    t_ba = nc.dram_tensor("ba_col", [DIM, 1], f32, kind="ExternalInput").ap()
    t_convm = nc.dram_tensor("convm", [DIM, KK * DIM], bf16, kind="ExternalInput").ap()
    t_convb = nc.dram_tensor("convb_col", [DIM, 1], f32, kind="ExternalInput").ap()
    t_woT = nc.dram_tensor("woT", [2 * DIM, 2 * DIM], f32, kind="ExternalInput").ap()
    t_bo = nc.dram_tensor("bo_col", [2 * DIM, 1], f32, kind="ExternalInput").ap()
    t_wiT = nc.dram_tensor("wiT", [2 * DIM, 2], f32, kind="ExternalInput").ap()
    t_bi = nc.dram_tensor("bi_col", [2, 1], f32, kind="ExternalInput").ap()
    t_ones = nc.dram_tensor("ones_row", [1, DIM], bf16, kind="ExternalInput").ap()
    t_wmask = nc.dram_tensor("wmask", [128, wpad], bf16, kind="ExternalInput").ap()
    t_out = nc.dram_tensor("out", [2, 1], f32, kind="ExternalOutput").ap()

    rg = [list(range(ncores))]

    with tile.TileContext(nc) as tc:
        with (
            tc.tile_pool(name="persist", bufs=1) as persist,
            tc.tile_pool(name="tabp", bufs=4) as tabp,
            tc.tile_pool(name="ohp", bufs=4) as ohp,
            tc.tile_pool(name="ohup", bufs=2) as ohup,
            tc.tile_pool(name="slabp", bufs=6) as slabp,
            tc.tile_pool(name="hsamp", bufs=2) as hsamp,
            tc.tile_pool(name="smallp", bufs=10) as smallp,
            tc.tile_pool(name="catp", bufs=3) as catp,
            tc.tile_pool(name="scrp", bufs=2) as scrp,
            tc.tile_pool(name="dram", bufs=1, space="DRAM") as dram,
            tc.tile_pool(name="ps4", bufs=1, space="PSUM") as ps4,
            tc.tile_pool(name="ps_x", bufs=2, space="PSUM") as ps_x,
            tc.tile_pool(name="ps_s", bufs=2, space="PSUM") as ps_s,
        ):
            # ---- small weights into SBUF ----
            wgT = persist.tile([DIM, DIM], f32, tag="wgT")
            bg_row = persist.tile([1, DIM], f32, tag="bg_row")
            waT = persist.tile([DIM, DIM], bf16, tag="waT")
            ba_col = persist.tile([DIM, 1], f32, tag="ba_col")
            convm = persist.tile([DIM, KK * DIM], bf16, tag="convm")
            convb_col = persist.tile([DIM, 1], f32, tag="convb_col")
            woT_sb = persist.tile([DIM, 4 * DIM], f32, tag="woT_sb")
            bo_sb = persist.tile([DIM, 2], f32, tag="bo_sb")
            wiT_sb = persist.tile([DIM, 4], f32, tag="wiT_sb")
            bi_sb = persist.tile([2, 1], f32, tag="bi_sb")
            ones_row = persist.tile([1, DIM], bf16, tag="ones_row")
            iota_col = persist.tile([128, 1], f32, tag="iota_col")
            wmask_sb = persist.tile([128, wpad], bf16, tag="wmask_sb")
            ident_bf = persist.tile([DIM, DIM], bf16, tag="ident_bf")
            ones_f32 = persist.tile([1, DIM], f32, tag="ones_f32")
            zcol = persist.tile([1, DIM], bf16, tag="zcol")
            zrow = persist.tile([1, 512], bf16, tag="zrow")

            nc.sync.dma_start(wgT[:], t_wgT[:])
            nc.sync.dma_start(bg_row[:], t_bg[:])
            nc.sync.dma_start(waT[:], t_waT[:])
            nc.sync.dma_start(ba_col[:], t_ba[:])
            nc.sync.dma_start(convm[:], t_convm[:])
            nc.sync.dma_start(convb_col[:], t_convb[:])
            for j in range(2):
                for i in range(2):
                    nc.sync.dma_start(
                        woT_sb[:, (j * 2 + i) * DIM:(j * 2 + i + 1) * DIM],
                        t_woT[j * DIM:(j + 1) * DIM, i * DIM:(i + 1) * DIM])
                nc.sync.dma_start(bo_sb[:, j:j + 1], t_bo[j * DIM:(j + 1) * DIM, :])
                nc.sync.dma_start(wiT_sb[:, 2 * j:2 * j + 2], t_wiT[j * DIM:(j + 1) * DIM, :])
            nc.sync.dma_start(bi_sb[:], t_bi[:])
            nc.sync.dma_start(ones_row[:], t_ones[:])
            nc.sync.dma_start(iota_col[:], t_iota[:])
            nc.sync.dma_start(wmask_sb[:], t_wmask[:])
            make_identity(nc, ident_bf[:])
            nc.gpsimd.memset(ones_f32[:], 1.0)
            nc.gpsimd.memset(zcol[:], 0.0)
            nc.gpsimd.memset(zrow[:], 0.0)

            # ---- persistent state ----
            xs = persist.tile([128, local_a], f32, tag="xs")           # dim-major
            hs_all = [persist.tile([128, local_a], bf16, tag=f"hsall{r}",
                                   name=f"hsall{r}")
                      for r in range(ncores)]
            sfp_bc = persist.tile([128, local_a], f32, tag="sfp_bc")
            sw_bc = persist.tile([128, wpad], f32, tag="sw_bc")
            winv_bc = persist.tile([128, wpad], f32, tag="winv_bc")
            sfp_row = persist.tile([1, local_a], f32, tag="sfp_row")
            sw_row = persist.tile([1, wpad], f32, tag="sw_row")
            winv_row = persist.tile([1, wpad], f32, tag="winv_row")
            gsrt = persist.tile([128, wpad], bf16, tag="gsrt")
            g_am = persist.tile([128, wpad], bf16, tag="g_am")
            imgA = persist.tile([128, wpad], bf16, tag="imgA")
            imgB = persist.tile([128, wpad], bf16, tag="imgB")
            hs_pT = persist.tile([128, local_w], bf16, tag="hspT")
            w_bf = persist.tile([1, local_w], bf16, tag="wbf")
            comp_sum = persist.tile([128, 1], f32, tag="csum")
            ys_ar = persist.tile([128, 1], f32, tag="ysar")

            nc.sync.dma_start(sfp_row[:], t_sfp[:])
            nc.sync.dma_start(sw_row[:], t_sw[:])
            nc.sync.dma_start(winv_row[:], t_winv[:])

            # ---- collective DRAM buffers (fresh set per rep) ----
            comm = {}

            def alloc_comm(rep):
                agh_in, agh_out = [], []
                for l in range(LAYER_GNN):
                    agh_in.append(dram.tile([128, local_a], bf16,
                                            tag=f"aghi{rep}_{l}",
                                            name=f"aghi{rep}_{l}"))
                    agh_out.append(dram.tile([128 * ncores, local_a], bf16,
                                             tag=f"agho{rep}_{l}",
                                             name=f"agho{rep}_{l}",
                                             addr_space="Shared"))
                comm["agh_in"] = agh_in
                comm["agh_out"] = agh_out
                comm["arc_in"] = dram.tile([128, 1], f32, tag=f"arci{rep}",
                                           name=f"arci{rep}")
                comm["arc_out"] = dram.tile([128, 1], f32, tag=f"arco{rep}",
                                            name=f"arco{rep}", addr_space="Shared")
                comm["arp_in"] = dram.tile([128, 1], f32, tag=f"arpi{rep}",
                                           name=f"arpi{rep}")
                comm["arp_out"] = dram.tile([128, 1], f32, tag=f"arpo{rep}",
                                            name=f"arpo{rep}", addr_space="Shared")

            def bcast_row(row, dst, width):
                o = 0
                while o < width:
                    ms = min(512, width - o)
                    pb = ps4.tile([128, 512], f32, tag=f"b{(o // 512) % nblk_a}")
                    nc.tensor.matmul(pb[:, :ms], ones_f32[:], row[:, o:o + ms],
                                     start=True, stop=True)
                    nc.vector.tensor_copy(dst[:, o:o + ms], pb[:, :ms])
                    o += ms

            def onehot_gather(tab_ap, n_chunks, starts, W, bc, n_loc, nblk, dma_eng):
                """Accumulate one-hot gather into psum blocks; returns them."""
                pbs = []
                for b in range(nblk):
                    bw = min(512, n_loc - b * 512)
                    if b < nblk_a:
                        pt = ps4.tile([128, 512], f32, tag=f"b{b}")
                    else:
                        pt = ps_x.tile([128, 512], f32, tag="x")
                    pbs.append((pt, bw))
                    nc.tensor.matmul(pt[:, :bw], zcol[:], zrow[:, :bw],
                                     start=True, stop=False)
                for g in range(0, n_chunks, 8):
                    bs = min(8, n_chunks - g)
                    slab = tabp.tile([128, 8 * 128], bf16, tag="tab")
                    dma_eng.dma_start(slab[:, :bs * 128],
                                      tab_ap[:, g * 128:(g + bs) * 128])
                    for j in range(bs):
                        k = g + j
                        ck = starts[k]
                        we = min(W, n_loc - ck)
                        oh = ohp.tile([128, W_W], bf16, tag="oh")
                        nc.vector.tensor_scalar(
                            oh[:, :we], bc[:, ck:ck + we], float(128 * k),
                            iota_col[:], op0=SUB, op1=EQ)
                        for (b, lo, hi) in _segments(ck, W, n_loc):
                            nc.tensor.matmul(
                                pbs[b][0][:, lo - 512 * b:hi - 512 * b],
                                slab[:, j * 128:(j + 1) * 128],
                                oh[:, lo - ck:hi - ck],
                                start=False, stop=False)
                for (pt, bw) in pbs:
                    nc.tensor.matmul(pt[:, :bw], zcol[:], zrow[:, :bw],
                                     start=False, stop=True)
                return pbs

            def fp_gather():
                bcast_row(sfp_row, sfp_bc, local_a)
                pbs = onehot_gather(t_etab, nch_f, fstarts, W_F, sfp_bc,
                                    local_a, nblk_a, nc.scalar)
                for b, (pt, bw) in enumerate(pbs):
                    nc.vector.tensor_copy(xs[:, b * 512:b * 512 + bw], pt[:, :bw])

            def word_gather():
                bcast_row(sw_row, sw_bc, wpad)
                bcast_row(winv_row, winv_bc, wpad)
                pbs = onehot_gather(t_wtab, nch_w, wstarts, W_W, sw_bc,
                                    wpad, nblk_w, nc.scalar)
                for b, (pt, bw) in enumerate(pbs):
                    nc.vector.tensor_copy(gsrt[:, b * 512:b * 512 + bw], pt[:, :bw])
                # transpose sorted gather to position-major chunks
                for s in range(wch):
                    pt = ps_s.tile([128, 128], bf16, tag="s")
                    nc.tensor.transpose(pt[:], gsrt[:, s * 128:(s + 1) * 128],
                                        ident_bf[:])
                    nc.vector.tensor_copy(g_am[:, s * 128:(s + 1) * 128], pt[:])
                # unsort: img[d, j] = sum_s g_am[s, d] * (winv[j] == s)
                ipbs = []
                for b in range(nblk_w):
                    bw = min(512, wpad - b * 512)
                    if b < nblk_a:
                        pt = ps4.tile([128, 512], f32, tag=f"b{b}")
                    else:
                        pt = ps_x.tile([128, 512], f32, tag="x")
                    ipbs.append((pt, bw))
                for kc in range(wch):
                    ohu = ohup.tile([128, wpad], bf16, tag="ohu")
                    nc.vector.tensor_scalar(ohu[:], winv_bc[:], float(128 * kc),
                                            iota_col[:], op0=SUB, op1=EQ)
                    for b, (pt, bw) in enumerate(ipbs):
                        nc.tensor.matmul(pt[:, :bw],
                                         g_am[:, kc * 128:(kc + 1) * 128],
                                         ohu[:, b * 512:b * 512 + bw],
                                         start=(kc == 0), stop=(kc == wch - 1))
                for b, (pt, bw) in enumerate(ipbs):
                    nc.vector.tensor_copy(imgA[:, b * 512:b * 512 + bw], pt[:, :bw])

            bmat_r = t_bmat.rearrange("(t p) m -> t p m", p=128)
            ach = na // 128

            def gnn_hs_ag(layer):
                hs_am = hsamp.tile([128, local_a], bf16, tag="hsam")
                for o in range(local_a // 128):
                    hp = ps_s.tile([128, 128], f32, tag="s")
                    nc.tensor.matmul(hp[:], ones_f32[:], bg_row[:],
                                     start=True, stop=False)
                    nc.tensor.matmul(hp[:], xs[:, o * 128:(o + 1) * 128], wgT[:],
                                     start=False, stop=True)
                    nc.scalar.activation(hs_am[:, o * 128:(o + 1) * 128], hp[:], Relu)
                nc.sync.dma_start(comm["agh_in"][layer][:], hs_am[:])
                nc.gpsimd.collective_compute(
                    "AllGather", mybir.AluOpType.bypass,
                    ins=[comm["agh_in"][layer][:].opt()],
                    outs=[comm["agh_out"][layer][:].opt()],
                    replica_groups=rg)
                for r in range(ncores):
                    nc.sync.dma_start(hs_all[r][:],
                                      comm["agh_out"][layer][r * 128:(r + 1) * 128, :])

            def gnn_spmm(layer):
                psums = [ps4.tile([128, 512], f32, tag=f"b{b}",
                                  name=f"spmm_l{layer}_{b}")
                         for b in range(nblk_a)]
                for ki in range(ach):
                    if ki % 2 == 0:
                        slab = slabp.tile([128, 2 * local_a], f8, tag="slab")
                        nc.sync.dma_start(slab[:, :local_a], bmat_r[ki])
                        nc.sync.dma_start(slab[:, local_a:], bmat_r[ki + 1])
                    part = slab[:, (ki % 2) * local_a:(ki % 2 + 1) * local_a]
                    lhsT = hs_all[ki // (local_a // 128)][
                        :, (ki % (local_a // 128)) * 128:(ki % (local_a // 128) + 1) * 128]
                    for b in range(nblk_a):
                        nc.tensor.matmul(psums[b][:], lhsT, part[:, b * 512:(b + 1) * 512],
                                         start=(ki == 0), stop=(ki == ach - 1))
                if layer < LAYER_GNN - 1:
                    for b in range(nblk_a):
                        nc.vector.tensor_tensor(xs[:, b * 512:(b + 1) * 512],
                                                xs[:, b * 512:(b + 1) * 512],
                                                psums[b][:], op=ADD)
                else:
                    # comp partial = sum_cols(xs2_local) + sum_cols(delta3_local)
                    acc = smallp.tile([128, 1], f32, tag="small")
                    tmp = smallp.tile([128, 1], f32, tag="small")
                    nc.vector.reduce_sum(acc[:], xs[:], axis=mybir.AxisListType.X)
                    for b in range(nblk_a):
                        nc.vector.reduce_sum(tmp[:], psums[b][:],
                                             axis=mybir.AxisListType.X)
                        nc.vector.tensor_tensor(acc[:], acc[:], tmp[:], op=ADD)
                    nc.sync.dma_start(comm["arc_in"][:], acc[:])
                    nc.gpsimd.collective_compute(
                        "AllReduce", mybir.AluOpType.add,
                        ins=[comm["arc_in"][:].opt()], outs=[comm["arc_out"][:].opt()],
                        replica_groups=rg)
                    nc.sync.dma_start(comp_sum[:], comm["arc_out"][:])

            def protein_conv():
                bufs = [imgA, imgB]
                for l in range(LAYER_CNN):
                    lo = WINDOW * (l + 1)
                    hi = lwin - WINDOW * (l + 1)
                    src, dst = bufs[l % 2], bufs[(l + 1) % 2]
                    o = lo
                    while o < hi:
                        ms = min(512, hi - o)
                        pc = ps_x.tile([128, 512], f32, tag="x")
                        for a in range(KK):
                            nc.tensor.matmul(
                                pc[:, :ms], convm[:, a * DIM:(a + 1) * DIM],
                                src[:, o + a - WINDOW:o + a - WINDOW + ms],
                                start=(a == 0), stop=(a == KK - 1))
                        nc.scalar.activation(dst[:, o:o + ms], pc[:, :ms], Relu,
                                             bias=convb_col[:])
                        o += ms
                    if l < LAYER_CNN - 1:
                        nc.vector.tensor_tensor(dst[:, lo:hi], dst[:, lo:hi],
                                                wmask_sb[:, lo:hi], op=MUL)

            def protein_hsp():
                xsp = [imgA, imgB][LAYER_CNN % 2]
                o = 0
                while o < local_w:
                    ms = min(512, local_w - o)
                    pc = ps_x.tile([128, 512], f32, tag="x")
                    nc.tensor.matmul(pc[:, :ms], waT[:], xsp[:, HALO + o:HALO + o + ms],
                                     start=True, stop=True)
                    nc.scalar.activation(hs_pT[:, o:o + ms], pc[:, :ms], Relu,
                                         bias=ba_col[:])
                    o += ms

            def tail():
                comp_bf = smallp.tile([128, 1], bf16, tag="smallbf")
                nc.vector.tensor_scalar_mul(comp_bf[:], comp_sum[:], 1.0 / na)
                ph = ps_s.tile([128, 128], f32, tag="s")
                nc.tensor.matmul(ph[:, :1], waT[:], comp_bf[:], start=True, stop=True)
                h_bf = smallp.tile([128, 1], bf16, tag="smallbf")
                nc.scalar.activation(h_bf[:], ph[:, :1], Relu, bias=ba_col[:])
                # w = tanh(h . hs_p)
                o = 0
                while o < local_w:
                    ms = min(512, local_w - o)
                    pw = ps_x.tile([128, 512], f32, tag="x")
                    nc.tensor.matmul(pw[:1, :ms], h_bf[:], hs_pT[:, o:o + ms],
                                     start=True, stop=True)
                    nc.scalar.activation(w_bf[:, o:o + ms], pw[:1, :ms], Tanh)
                    o += ms
                # ys partial = sum_l w[l] * hs_p[:, l]
                prev = None
                o = 0
                while o < local_w:
                    ms = min(512, local_w - o)
                    pb = ps_x.tile([128, 512], f32, tag="x")
                    nc.tensor.matmul(pb[:, :ms], ones_row[:], w_bf[:, o:o + ms],
                                     start=True, stop=True)
                    scr = scrp.tile([128, 512], f32, tag="scr")
                    nc.vector.tensor_tensor(scr[:, :ms], pb[:, :ms],
                                            hs_pT[:, o:o + ms], op=MUL)
                    acc = smallp.tile([128, 1], f32, tag="small")
                    nc.vector.reduce_sum(acc[:], scr[:, :ms], axis=mybir.AxisListType.X)
                    if prev is not None:
                        nc.vector.tensor_tensor(acc[:], acc[:], prev[:], op=ADD)
                    prev = acc
                    o += ms
                nc.sync.dma_start(comm["arp_in"][:], prev[:])
                nc.gpsimd.collective_compute(
                    "AllReduce", mybir.AluOpType.add,
                    ins=[comm["arp_in"][:].opt()], outs=[comm["arp_out"][:].opt()],
                    replica_groups=rg)
                nc.sync.dma_start(ys_ar[:], comm["arp_out"][:])
                # output MLP on cat = [compound, protein]
                cat = catp.tile([128, 2], f32, tag="cat")
                nc.vector.tensor_scalar_mul(cat[:, 0:1], comp_sum[:], 1.0 / na)
                nc.vector.tensor_scalar_mul(cat[:, 1:2], ys_ar[:], 1.0 / nw)
                for l in range(LAYER_OUT):
                    ncat = catp.tile([128, 2], f32, tag="cat")
                    for i in range(2):
                        pm = ps_s.tile([128, 128], f32, tag="s")
                        for j in range(2):
                            nc.tensor.matmul(pm[:, :1],
                                             woT_sb[:, (j * 2 + i) * DIM:(j * 2 + i + 1) * DIM],
                                             cat[:, j:j + 1],
                                             start=(j == 0), stop=(j == 1))
                        nc.scalar.activation(ncat[:, i:i + 1], pm[:, :1], Relu,
                                             bias=bo_sb[:, i:i + 1])
                    cat = ncat
                pf = ps_s.tile([128, 128], f32, tag="s")
                for j in range(2):
                    nc.tensor.matmul(pf[:2, :1], wiT_sb[:, 2 * j:2 * j + 2],
                                     cat[:, j:j + 1], start=(j == 0), stop=(j == 1))
                res = smallp.tile([2, 1], f32, tag="res")
                nc.scalar.activation(res[:], pf[:2, :1], Ident, bias=bi_sb[:])
                nc.sync.dma_start(t_out[:], res[:])

            def finish_early():
                res2 = smallp.tile([2, 1], f32, tag="res")
                nc.vector.tensor_copy(res2[:], bi_sb[:])
                nc.sync.dma_start(t_out[:], res2[:])

            for rep in range(reps):
                alloc_comm(rep)
                if stage >= 1:
                    fp_gather()
                if stage >= 3:
                    gnn_hs_ag(0)
                if stage >= 2:
                    word_gather()
                if stage >= 3:
                    gnn_spmm(0)
                if stage >= 5:
                    gnn_hs_ag(1)
                if stage >= 4:
                    protein_conv()
                if stage >= 5:
                    gnn_spmm(1)
                if stage >= 6:
                    gnn_hs_ag(2)
                if stage >= 7:
                    protein_hsp()
                if stage >= 6:
                    gnn_spmm(2)
                if stage >= 7:
                    tail()
                else:
                    finish_early()

    nc.compile()
    return nc


def prep_in_maps(inputs, na, nw, nfp, nword, ncores):
    """Host-side sharding/layout prep (layout + lossless/near-lossless casts)."""
    local_a = na // ncores
    local_w = nw // ncores
    lwin = local_w + 2 * HALO
    wch = _ceil_div(lwin, 128)
    wpad = wch * 128
    tpad_f = _ceil_div(nfp, 128) * 128
    tpad_w = _ceil_div(nword + 1, 128) * 128
    fstarts = _win_starts(tpad_f // 128, local_a, nfp, W_F)
    wstarts = _win_starts(tpad_w // 128, wpad, nword, W_W)

    fingerprints = np.asarray(inputs["fingerprints"]).astype(np.int64)
    adjacency = np.asarray(inputs["adjacency"], dtype=np.float32)
    words = np.asarray(inputs["words"]).astype(np.int64)
    embed_fp = np.asarray(inputs["embed_fp"], dtype=np.float32)
    embed_word = np.asarray(inputs["embed_word"], dtype=np.float32)
    Wg = np.asarray(inputs["Wg"], dtype=np.float32)
    bg = np.asarray(inputs["bg"], dtype=np.float32)
    conv_k = np.asarray(inputs["conv_k"], dtype=np.float32)
    conv_b = np.asarray(inputs["conv_b"], dtype=np.float32)
    Wa = np.asarray(inputs["Wa"], dtype=np.float32)
    ba = np.asarray(inputs["ba"], dtype=np.float32)
    Wo = np.asarray(inputs["Wo"], dtype=np.float32)
    bo = np.asarray(inputs["bo"], dtype=np.float32)
    Wi = np.asarray(inputs["Wi"], dtype=np.float32)
    bi = np.asarray(inputs["bi"], dtype=np.float32)

    # ---- per-core fp sort; permutation absorbed into adjacency relayout ----
    orders = []
    perm = np.empty(na, np.int64)
    for c in range(ncores):
        sl = fingerprints[c * local_a:(c + 1) * local_a]
        o = np.argsort(sl, kind="stable")
        orders.append(o)
        perm[c * local_a:(c + 1) * local_a] = c * local_a + o

    # verify one-hot windows cover the sorted data
    for c in range(ncores):
        s = np.sort(fingerprints[c * local_a:(c + 1) * local_a])
        for k in range(tpad_f // 128):
            lo_i = np.searchsorted(s, 128 * k, "left")
            hi_i = np.searchsorted(s, min(128 * (k + 1), nfp), "left")
            if hi_i > lo_i:
                ck = fstarts[k]
                assert lo_i >= ck and hi_i <= ck + W_F, \
                    f"fp window overflow core {c} chunk {k}"

    # adjacency -> fp8 bit pattern, both axes permuted to sorted order
    a8 = (adjacency != 0).astype(np.uint8) * np.uint8(0x38)
    a8 = a8[perm][:, perm]

    # partition-major bf16 tables: tab_pm[p, t*128+d] = tab[t*128+p, d]
    def pm_table(tab, tpad):
        t = np.zeros((tpad, DIM), np.float32)
        t[:tab.shape[0]] = tab
        nch = tpad // 128
        return np.ascontiguousarray(
            t.reshape(nch, 128, DIM).transpose(1, 0, 2).reshape(128, nch * DIM)
        ).astype(BF16)

    etab_pm = pm_table(embed_fp, tpad_f)
    wtab_full = np.concatenate([embed_word, np.zeros((1, DIM), np.float32)], axis=0)
    wtab_pm = pm_table(wtab_full, tpad_w)

    K2 = conv_k[0, 0]
    M = np.zeros((DIM, KK * DIM), np.float32)
    for a in range(KK):
        Ma = np.zeros((DIM, DIM), np.float32)
        for b_ in range(KK):
            Ma += K2[a, b_] * np.eye(DIM, k=5 - b_, dtype=np.float32)
        M[:, a * DIM:(a + 1) * DIM] = Ma

    common = dict(
        etabpm=etab_pm,
        wtabpm=wtab_pm,
        iota_col=np.arange(128, dtype=np.float32).reshape(128, 1),
        wgT=np.ascontiguousarray(Wg.T).astype(np.float32),
        bg_row=bg.reshape(1, DIM).astype(np.float32),
        waT=np.ascontiguousarray(Wa.T).astype(BF16),
        ba_col=ba.reshape(DIM, 1).astype(np.float32),
        convm=M.astype(BF16),
        convb_col=np.full((DIM, 1), conv_b[0], np.float32),
        woT=np.ascontiguousarray(Wo.T).astype(np.float32),
        bo_col=bo.reshape(2 * DIM, 1).astype(np.float32),
        wiT=np.ascontiguousarray(Wi.T).astype(np.float32),
        bi_col=bi.reshape(2, 1).astype(np.float32),
        ones_row=np.ones((1, DIM), BF16),
    )

    in_maps = []
    for c in range(ncores):
        sl = slice(c * local_a, (c + 1) * local_a)
        bmat = np.ascontiguousarray(a8[sl, :].T).view(F8)
        sfp = np.sort(fingerprints[sl]).astype(np.float32).reshape(1, local_a)

        # word window: values (OOB/pad -> nword sentinel), sorted
        w0 = c * local_w - HALO
        pos = np.arange(wpad)
        gidx = w0 + pos
        valid = (gidx >= 0) & (gidx < nw) & (pos < lwin)
        widx = np.where(valid, words[np.clip(gidx, 0, nw - 1)], nword).astype(np.int64)
        order = np.argsort(widx, kind="stable")
        swidx = widx[order]
        # verify windows cover real words
        for k in range(tpad_w // 128):
            lo_i = np.searchsorted(swidx, 128 * k, "left")
            hi_i = np.searchsorted(swidx, min(128 * (k + 1), nword), "left")
            if hi_i > lo_i:
                ck = wstarts[k]
                assert lo_i >= ck and hi_i <= ck + W_W, \
                    f"word window overflow core {c} chunk {k}"
        # unsort index: winv[j] = sorted position of window position j
        # (OOB positions get an out-of-range index -> img column stays zero)
        winv = np.full(wpad, wpad + 128, np.float32)
        inv = np.empty(wpad, np.int64)
        inv[order] = np.arange(wpad)
        winv[valid] = inv[valid].astype(np.float32)

        wmask = np.broadcast_to(
            ((gidx >= 0) & (gidx < nw)).astype(BF16)[None, :], (128, wpad))
        m = dict(common)
        m.update(bmat=bmat,
                 sfp_row=sfp,
                 sw_row=swidx.astype(np.float32).reshape(1, wpad),
                 winv_row=winv.reshape(1, wpad),
                 wmask=np.ascontiguousarray(wmask))
        in_maps.append(m)
    return in_maps


_CACHE = {}


def _get_kernel(cfg_key):
    if cfg_key not in _CACHE:
        na, nw, nfp, nword, ncores = cfg_key
        _CACHE[cfg_key] = build_kernel(na, nw, nfp, nword, ncores)
    return _CACHE[cfg_key]


def kernel(**inputs) -> np.ndarray:
    from concourse import bass_utils
    cfg = FULL
    key = (cfg["na"], cfg["nw"], cfg["nfp"], cfg["nword"], cfg["ncores"])
    nc = _get_kernel(key)
    in_maps = prep_in_maps(inputs, *key)
    res = bass_utils.run_bass_kernel_spmd(
        nc, in_maps, core_ids=list(range(cfg["ncores"])), trace=False)
    out = np.asarray(res.results[0]["out"], np.float32).reshape(1, 2)
    return out
